# revision 25
# baseline (speedup 1.0000x reference)
"""Trainium2 Bass kernel for nn_MemristorCNN (embedding_lookup, 8 cores).

Strategy (per sharding hint):
- Host gathers the codebook weight W1 = values[w_idx1] and ships the
  *gathered weight* in bf16, column-sharded over in_features (12544
  features = 4 conv2 output channels per core), laid out [128, 99, 512]
  so the whole 12.9 MB stream is ONE DMA instruction (issue-rate, not
  bandwidth, limited the old 98-DMA stream); tile k=98 carries
  fc1_bias/8 against an on-device ones-column so the bias rides the
  PSUM accumulation and survives the ReduceScatter sum.
- Conv stack runs data-parallel (4 images per core); conv1 packs
  (tap, half-image) into K=72 with dx pre-shifted on host; conv2 packs
  (image, channel, dx-pair) into K=128 with 6 tap passes.
- AllToAll redistributes conv output h from batch-sharded to
  feature-sharded with an img-major payload so the receive buffer is a
  contiguous [32, 12544] view; ONE xbar transpose-DMA (out[p,k,i] =
  in[i, 128k+p]) lands it feature-major in SBUF, replacing 98 PE
  transposes; fc1 accumulates 99 matmuls; ReduceScatter sums partials;
  relu + fc2 (DVE mult+reduce) finish on device and the host
  concatenates the per-core [4, 4] outputs.
"""

import sys

import numpy as np
import ml_dtypes

BF16NP = ml_dtypes.bfloat16

for _p in ("/opt/trn_rl_repo",):
    if _p not in sys.path:
        sys.path.insert(0, _p)

import concourse.bacc as bacc
import concourse.bass as bass  # noqa: F401
import concourse.tile as tile
from concourse import mybir
from concourse.bass_utils import run_bass_kernel_spmd

F32 = mybir.dt.float32
BF16 = mybir.dt.bfloat16
RELU = mybir.ActivationFunctionType.Relu
COPY = mybir.ActivationFunctionType.Copy

N_CORES = 8
B = 32
IMG = 224
C1, C2 = 16, 32
PH, PW = 112, 112
HH, HW = 56, 56
FEAT = C2 * HH * HW          # 100352
FSH = FEAT // N_CORES        # 12544
NK = FSH // 128              # 98
NKB = NK + 1                 # +1 bias tile
H1 = 512
NOUT = 4

_CACHE = {}


def _build_program(stop_after: str = 'full'):
    nc = bacc.Bacc("TRN2", target_bir_lowering=False, debug=False,
                   num_devices=N_CORES)
    _emit(nc, stop_after)
    nc.compile()
    return nc


def _emit(nc, stop_after: str):
    # ---- kernel I/O ----
    x9_t = nc.dram_tensor("x9", [72, PH, 232], BF16, kind="ExternalInput")
    s1_t = nc.dram_tensor("s1", [72, 128], BF16, kind="ExternalInput")
    s2_t = nc.dram_tensor("s2", [6, 128, 128], BF16, kind="ExternalInput")
    w1t_t = nc.dram_tensor("w1t", [128, NKB * H1], BF16, kind="ExternalInput")
    w2r_t = nc.dram_tensor("w2r", [4, NOUT, H1], F32, kind="ExternalInput")
    b2t_t = nc.dram_tensor("b2t", [4, 4], F32, kind="ExternalInput")
    cb1_t = nc.dram_tensor("cb1", [128, 1], F32, kind="ExternalInput")
    cb2_t = nc.dram_tensor("cb2", [128, 1], F32, kind="ExternalInput")
    if stop_after in ("dumphT", "dumpw", "dumpfc1"):
        shp = {"dumphT": [128, NKB, 32], "dumpw": [128, NKB, H1],
               "dumpfc1": [B, H1]}[stop_after]
        out_t = nc.dram_tensor("out", shp, F32, kind="ExternalOutput")
    elif stop_after in ("dumpa2ain", "dumpa2aout"):
        out_t = nc.dram_tensor("out", [B, FSH], BF16, kind="ExternalOutput")
    elif stop_after == "dumph":
        out_t = nc.dram_tensor("out", [128, 3136], BF16, kind="ExternalOutput")
    else:
        out_t = nc.dram_tensor("out", [4, NOUT], F32, kind="ExternalOutput")

    # ---- internal DRAM (collective bounce buffers) ----
    a2a_in = nc.dram_tensor("a2a_in", [B, FSH], BF16)
    a2a_out = nc.dram_tensor("a2a_out", [N_CORES, 4, FSH], BF16)
    rs_in = nc.dram_tensor("rs_in", [B, H1], F32)
    rs_out = nc.dram_tensor("rs_out", [4, H1], F32)

    groups = [list(range(N_CORES))]

    with tile.TileContext(nc) as tc:
        with (
            tc.tile_pool(name="const", bufs=1) as cpool,
            tc.tile_pool(name="ps", bufs=4, space="PSUM") as pspool,
            tc.tile_pool(name="work", bufs=2) as wkpool,
            tc.tile_pool(name="xin", bufs=2) as xpool,
            tc.tile_pool(name="persist", bufs=1) as pers,
        ):
            # -------- latency-critical loads first --------
            # conv1 input: partition (dy*3+dx)*8 + h holds
            # x_pad[img(h), y0(h)+dy+y, dx+c]; row-eighths triple-buffered.
            x9_tiles = []
            for e in range(8):
                x9e = xpool.tile([72, 14, 232], BF16, tag="x9")
                if e == 0:
                    nc.sync.dma_start(out=x9e[:, :, :],
                                      in_=x9_t[:, 0:14, :])
                x9_tiles.append(x9e)
            s1_sb = cpool.tile([72, 128], BF16, tag="s1")
            nc.sync.dma_start(out=s1_sb[:, :], in_=s1_t[:, :])
            for e in range(1, 8):
                nc.sync.dma_start(out=x9_tiles[e][:, :, :],
                                  in_=x9_t[:, 14 * e:14 * e + 14, :])

            # small constants first on the scalar ring (FIFO per ring —
            # anything queued after the big w1t DMA drains after it)
            cb1_sb = cpool.tile([128, 1], F32, tag="cb1")
            nc.scalar.dma_start(out=cb1_sb[:, :], in_=cb1_t[:, :])
            cb2_sb = cpool.tile([128, 1], F32, tag="cb2")
            nc.scalar.dma_start(out=cb2_sb[:, :], in_=cb2_t[:, :])
            s2_sb = cpool.tile([128, 6, 128], BF16, tag="s2")
            nc.scalar.dma_start(out=s2_sb[:, :, :],
                                in_=s2_t[:, :, :].rearrange("t p m -> p t m"))

            w2r_sb = cpool.tile([4, NOUT, H1], F32, tag="w2r")
            nc.scalar.dma_start(out=w2r_sb[:, :, :], in_=w2r_t[:, :, :])
            b2t_sb = cpool.tile([4, 4], F32, tag="b2t")
            nc.scalar.dma_start(out=b2t_sb[:, :], in_=b2t_t[:, :])

            # fc1 weight stream: 13 chunked DMAs on the idle GpSimd
            # (SWDGE) queue — keeps per-descriptor size ~8 KB so the SDMA
            # engines round-robin fairly with the latency-critical x9 /
            # repack traffic (one huge descriptor starves them), and
            # keeps the issue cost off the ACT/SP rings.
            WCH = 8                         # k-tiles per chunk
            wch_edges = list(range(0, NKB, WCH)) + [NKB]
            if stop_after not in ("dumpa2ain", "dumpa2aout"):
                wts = pers.tile([128, NKB, H1], BF16, tag="w1")
                wflat = wts[:, :, :].rearrange("p k m -> p (k m)")
                for k0, k1 in zip(wch_edges[:2], wch_edges[1:3]):
                    nc.gpsimd.dma_start(out=wflat[:, H1 * k0:H1 * k1],
                                        in_=w1t_t[:, H1 * k0:H1 * k1])

            # conv2 input buffer: partition e*64 + img*16 + ch holds the
            # padded channel image, dx-shifted by e.  Repack fills rows
            # 1..112 full-width; only rows 0/113 need zeroing.
            c2in = pers.tile([128, 114, 117], BF16, tag="bigC")
            nc.gpsimd.memset(c2in[:, 0, :], 0.0)
            nc.gpsimd.memset(c2in[:, 113, :], 0.0)

            # pool1 rows are 117 wide with zeroed borders (cols 0,
            # 113-116 + one spare element) so the repack shifts by e via
            # a single contiguous flat copy per (chunk, half, e).
            pool1_a = pers.tile([128, 28 * 117 + 1], BF16, tag="bigB1")
            pool1_b = pers.tile([128, 28 * 117 + 1], BF16, tag="bigB2")
            pool1_parts = [pool1_a, pool1_b]
            pool1_views = []
            for t in pool1_parts:
                pv = t[:, 0:28 * 117].rearrange("p (r c) -> p r c", c=117)
                nc.gpsimd.memset(pv[:, :, 0], 0.0)
                nc.gpsimd.memset(pv[:, :, 113:117], 0.0)
                nc.gpsimd.memset(t[:, 28 * 117:], 0.0)
                pool1_views.append(pv)

            # fc1 bias rides k-tile 98: ones column on partition 0
            hT = pers.tile([128, NKB, 32], BF16, tag="bigHT")
            nc.gpsimd.memset(hT[:, NK, :], 0.0)
            nc.gpsimd.memset(hT[0:1, NK, :], 1.0)

            # rest of the fc1 weight chunks
            if stop_after not in ("dumpa2ain", "dumpa2aout"):
                for k0, k1 in zip(wch_edges[2:-1], wch_edges[3:]):
                    nc.gpsimd.dma_start(out=wflat[:, H1 * k0:H1 * k1],
                                        in_=w1t_t[:, H1 * k0:H1 * k1])

            # ---------------- conv1 + pool1 + relu ----------------
            # out partition m = h*16 + oc = half*64 + img*16 + oc
            for T in range(28):            # 2 pooled rows per psum tile
                ps = pspool.tile([128, 2, 512], F32, tag="ps")
                for g in range(2):
                    yp = T * 2 + g         # pooled row within half
                    e, ypl = yp // 7, yp % 7
                    nc.tensor.matmul(
                        ps[:, g, 0:448],
                        lhsT=s1_sb[:, :],
                        rhs=x9_tiles[e][:, 2 * ypl:2 * ypl + 2, :224],
                        start=True, stop=True)
                v = ps[:, :, 0:448].rearrange("p g (r x w) -> p g r x w",
                                              r=2, w=2)
                c1 = wkpool.tile([128, 2, 2, 112], F32, tag="mc")
                nc.scalar.activation(c1[:, :, :, :], v[:, :, :, :, 1], COPY)
                m1 = wkpool.tile([128, 2, 2, 112], F32, tag="mx")
                nc.vector.tensor_max(m1[:, :, :, :], v[:, :, :, :, 0],
                                     c1[:, :, :, :])
                m2 = wkpool.tile([128, 2, 112], F32, tag="mxb")
                nc.vector.tensor_max(m2[:, :, :], m1[:, :, 0, :],
                                     m1[:, :, 1, :])
                half_t, row_t = divmod(2 * T, 28)
                nc.scalar.activation(
                    pool1_views[half_t][:, row_t:row_t + 2, 1:113],
                    m2[:, :, :], RELU, bias=cb1_sb[:, :])

            if stop_after == "conv1":
                dbg = wkpool.tile([4, NOUT], F32, tag="outsb")
                nc.vector.tensor_copy(dbg[:, :], pool1_views[0][0:4, 0, 1:5])
                nc.sync.dma_start(out=out_t[:, :], in_=dbg[:, :])
                return

            # -------- repack pool1 -> conv2 input (padded, merged halves,
            # two dx-shifted copies); 8 flat contiguous DMAs (one
            # descriptor per partition).  c2in[R0+r, c] = pool[r, c+e];
            # the pool's zeroed borders supply the conv padding.
            c2flat = c2in[:, :, :].rearrange("p r c -> p (r c)")
            for chunk in range(2):
                for half in range(2):
                    r0 = 117 * (56 * half + 1 + 28 * chunk)
                    for e in range(2):
                        nc.sync.dma_start(
                            out=c2flat[64 * e:64 * e + 64,
                                       r0:r0 + 28 * 117],
                            in_=pool1_parts[chunk][64 * half:64 * half + 64,
                                                   e:e + 28 * 117])

            # ---------------- conv2 + pool2 + relu ----------------
            # out partition m = img*32 + oc; 6 passes t=(dy, grp):
            # partition block e supplies tap dx = 2*grp + e.
            h_sb = pers.tile([128, 7, 4, 2, 56], BF16, tag="bigD")
            for T in range(14):            # 8 conv rows / 4 pooled rows
                ps = pspool.tile([128, 2, 512], F32, tag="ps")
                for sub in range(2):
                    y0 = 8 * T + 4 * sub
                    for t in range(6):
                        dy, grp = t // 2, t % 2
                        nc.tensor.matmul(
                            ps[:, sub, 0:448],
                            lhsT=s2_sb[:, t, :],
                            rhs=c2in[:, y0 + dy:y0 + dy + 4,
                                     2 * grp:2 * grp + 112],
                            start=(t == 0), stop=(t == 5))
                v = ps[:, :, 0:448].rearrange("p s (r x w) -> p s r x w",
                                              r=4, w=2)
                c1 = wkpool.tile([128, 2, 4, 56], F32, tag="mc2")
                nc.scalar.activation(c1[:, :, :, :], v[:, :, :, :, 1], COPY)
                m1 = wkpool.tile([128, 2, 4, 56], F32, tag="mx2")
                nc.vector.tensor_max(m1[:, :, :, :], v[:, :, :, :, 0],
                                     c1[:, :, :, :])
                v2 = m1[:, :, :, :].rearrange("p s (rp w) x -> p s rp w x",
                                              w=2)
                m2 = wkpool.tile([128, 2, 2, 56], F32, tag="mxb2")
                nc.vector.tensor_max(m2[:, :, :, :], v2[:, :, :, 0, :],
                                     v2[:, :, :, 1, :])
                # pooled rows 4T..4T+4 -> h_sb[T//2, 2*(T%2) + (0..1), ...]
                nc.scalar.activation(
                    h_sb[:, T // 2, 2 * (T % 2):2 * (T % 2) + 2, :, :],
                    m2[:, :, :, :], RELU, bias=cb2_sb[:, :])

            if stop_after == "conv2":
                dbg = wkpool.tile([4, NOUT], F32, tag="outsb")
                nc.vector.tensor_copy(dbg[:, :], h_sb[0:4, 0, 0, 0, 0:4])
                nc.sync.dma_start(out=out_t[:, :], in_=dbg[:, :])
                return

            # -------- AllToAll: batch-shard -> feature-shard --------
            # payload row (4*dest + img_loc), col (ch_loc, sp): receiver
            # gets a contiguous [32, 12544] img-major view.  One DMA per
            # local image (DMA APs are limited to 3 dims).
            h_flat = h_sb[:, :, :, :, :].rearrange(
                "(i m) t j r x -> i m (t j r x)", i=4)
            for il in range(4):
                nc.sync.dma_start(
                    out=a2a_in[:, :].rearrange(
                        "(d i) (c s) -> i d c s", d=8, c=4)[il],
                    in_=h_flat[il])
            if stop_after == "dumph":
                nc.sync.dma_start(
                    out=out_t[:, :],
                    in_=h_sb[:, :, :, :, :].rearrange("p t j r x -> p (t j r x)"))
                return

            if stop_after == "dumpa2ain":
                dbg = pers.tile([32, FSH], BF16, tag="dmpa")
                nc.sync.dma_start(out=dbg[:, :], in_=a2a_in[:, :])
                nc.sync.dma_start(out=out_t[:, :], in_=dbg[:, :])
                return

            nc.gpsimd.collective_compute(
                "AllToAll", mybir.AluOpType.bypass, replica_groups=groups,
                ins=[a2a_in[:, :]], outs=[a2a_out[:, :, :]])

            if stop_after == "dumpa2aout":
                dbg = pers.tile([32, FSH], BF16, tag="dmpa")
                nc.sync.dma_start(
                    out=dbg[:, :],
                    in_=a2a_out[:, :, :].rearrange("s i f -> (s i) f"))
                nc.sync.dma_start(out=out_t[:, :], in_=dbg[:, :])
                return

            # -------- ONE xbar transpose-DMA -> feature-major hT --------
            # out[p, k, i] = in[i, 128k + p]
            nc.sync.dma_start(
                out=hT[:, 0:NK, :],
                in_=a2a_out[:, :, :].rearrange("s i f -> (s i) f"),
                transpose=True)

            if stop_after == "a2a":
                dbg = wkpool.tile([4, NOUT], F32, tag="outsb")
                nc.vector.tensor_copy(dbg[:, :], hT[0:4, 0, 0:4])
                nc.sync.dma_start(out=out_t[:, :], in_=dbg[:, :])
                return

            if stop_after in ("dumphT", "dumpw"):
                src = hT if stop_after == "dumphT" else wts
                n = 32 if stop_after == "dumphT" else H1
                for k in range(NKB):
                    dbg = wkpool.tile([128, n], F32, tag="dmp")
                    nc.vector.tensor_copy(dbg[:, :], src[:, k, :])
                    nc.sync.dma_start(out=out_t[:, k, :], in_=dbg[:, :])
                return

            # ---------------- fc1 partial ----------------
            fc1_ps = pspool.tile([32, H1], F32, tag="ps")
            for k in range(NKB):
                nc.tensor.matmul(fc1_ps[:, :], lhsT=hT[:, k, :],
                                 rhs=wts[:, k, :],
                                 start=(k == 0), stop=(k == NKB - 1))
            fc1_sb = wkpool.tile([B, H1], F32, tag="fc1")
            nc.vector.tensor_copy(fc1_sb[:, :], fc1_ps[:, :])
            nc.sync.dma_start(out=rs_in[:, :], in_=fc1_sb[:, :])

            if stop_after == "fc1":
                nc.sync.dma_start(out=out_t[:, :], in_=fc1_sb[0:4, 0:4])
                return

            # -------- ReduceScatter + relu + fc2 --------
            nc.gpsimd.collective_compute(
                "ReduceScatter", mybir.AluOpType.add, replica_groups=groups,
                ins=[rs_in[:, :]], outs=[rs_out[:, :]])

            h1 = wkpool.tile([4, H1], F32, tag="h1")
            nc.sync.dma_start(out=h1[:, :], in_=rs_out[:, :])
            nc.scalar.activation(h1[:, :], h1[:, :], RELU)

            if stop_after == "rs":
                dbg = wkpool.tile([4, NOUT], F32, tag="outsb")
                nc.vector.tensor_copy(dbg[:, :], h1[0:4, 0:4])
                nc.sync.dma_start(out=out_t[:, :], in_=dbg[:, :])
                return

            out_sb = wkpool.tile([4, NOUT], F32, tag="outsb")
            for o in range(NOUT):
                prod = wkpool.tile([4, H1], F32, tag="prod")
                nc.vector.tensor_mul(prod[:, :], h1[:, :], w2r_sb[:, o, :])
                nc.vector.tensor_reduce(
                    out_sb[:, o:o + 1], prod[:, :],
                    axis=mybir.AxisListType.X, op=mybir.AluOpType.add)
            nc.vector.tensor_add(out_sb[:, :], out_sb[:, :], b2t_sb[:, :])
            nc.sync.dma_start(out=out_t[:, :], in_=out_sb[:, :])


def _get_program():
    key = ("prog",)
    if key not in _CACHE:
        _CACHE[key] = _build_program()
    return _CACHE[key]


def _host_prep(x, conv1_w, conv1_b, conv2_w, conv2_b, values, w_idx1,
               fc1_b, w_idx2, fc2_b):
    """Build per-core input maps (numpy, bf16 for PE-facing tensors)."""
    f32 = np.float32
    x = np.asarray(x, f32)
    conv1_w = np.asarray(conv1_w, f32)
    conv2_w = np.asarray(conv2_w, f32)
    values = np.asarray(values, f32)
    w_idx1 = np.asarray(w_idx1)
    w_idx2 = np.asarray(w_idx2)

    x_pad = np.zeros((B, 226, 232), f32)
    x_pad[:, 1:225, 1:225] = x[:, 0]

    # x9[c]: [72, 112, 232]; partition (dy*3+dx)*8 + h, h = 4*half + img_loc
    x9 = np.zeros((N_CORES, 72, PH, 232), f32)
    for dy in range(3):
        for dx in range(3):
            for h in range(8):
                half, il = h // 4, h % 4
                y0 = PH * half
                for c in range(N_CORES):
                    x9[c, (dy * 3 + dx) * 8 + h, :, :232 - dx] = \
                        x_pad[4 * c + il, y0 + dy:y0 + dy + PH, dx:]

    s1 = np.zeros((72, 128), f32)
    for dy in range(3):
        for dx in range(3):
            for h in range(8):
                s1[(dy * 3 + dx) * 8 + h, 16 * h:16 * h + C1] = \
                    conv1_w[:, 0, dy, dx]

    # conv2 stationaries [6, 128, 128]: pass t = dy*2 + grp;
    # partition p = e*64 + img*16 + ch supplies tap dx = 2*grp + e
    s2 = np.zeros((6, 128, 128), f32)
    for t in range(6):
        dy, grp = t // 2, t % 2
        for e in range(2):
            dx = 2 * grp + e
            if dx > 2:
                continue
            for img in range(4):
                for ch in range(C1):
                    s2[t, 64 * e + 16 * img + ch, 32 * img:32 * img + C2] = \
                        conv2_w[:, ch, dy, dx]

    # fc1 weight [128, 99, 512]: tile k row p = feature 128k+p of the
    # core's shard; tile 98 row 0 = fc1_b/8 (ones-column bias trick).
    b1_8 = np.asarray(fc1_b, f32) / N_CORES
    w1ts = []
    for c in range(N_CORES):
        idx = w_idx1[:, FSH * c:FSH * (c + 1)]             # [512, 12544]
        wt = np.zeros((128, NKB, H1), f32)
        wt[:, :NK, :] = values[idx].T.reshape(NK, 128, H1).transpose(1, 0, 2)
        wt[0, NK, :] = b1_8
        w1ts.append(np.ascontiguousarray(
            wt.reshape(128, NKB * H1)).astype(BF16NP))

    w2 = np.ascontiguousarray(values[w_idx2]).astype(f32)     # [4, 512]
    w2r = np.broadcast_to(w2[None, :, :], (4, NOUT, H1)).copy()
    b2t = np.broadcast_to(np.asarray(fc2_b, f32), (4, 4)).copy()

    cb1 = np.zeros((128, 1), f32)
    for h in range(8):
        cb1[16 * h:16 * h + C1, 0] = np.asarray(conv1_b, f32)
    cb2 = np.zeros((128, 1), f32)
    for img in range(4):
        cb2[32 * img:32 * img + C2, 0] = np.asarray(conv2_b, f32)

    s1 = s1.astype(BF16NP)
    s2 = s2.astype(BF16NP)
    in_maps = []
    for c in range(N_CORES):
        in_maps.append({
            "x9": np.ascontiguousarray(x9[c]).astype(BF16NP),
            "s1": s1, "s2": s2,
            "w1t": w1ts[c],
            "w2r": w2r, "b2t": b2t,
            "cb1": cb1, "cb2": cb2,
        })
    return in_maps


def kernel(x, conv1_w, conv1_b, conv2_w, conv2_b, values, w_idx1, fc1_b,
           w_idx2, fc2_b, _trace=False, _trace_kwargs=None):
    nc = _get_program()
    in_maps = _host_prep(x, conv1_w, conv1_b, conv2_w, conv2_b, values,
                         w_idx1, fc1_b, w_idx2, fc2_b)
    res = run_bass_kernel_spmd(nc, in_maps, core_ids=list(range(N_CORES)),
                               trace=_trace, **(_trace_kwargs or {}))
    out = np.zeros((B, NOUT), np.float32)
    for c in range(N_CORES):
        out[4 * c:4 * c + 4] = res.results[c]["out"]
    if _trace:
        kernel.last_result = res
    return out


if __name__ == "__main__":
    rng = np.random.default_rng(0)
    ins = {
        "x": rng.standard_normal((B, 1, IMG, IMG), dtype=np.float32),
        "conv1_w": rng.standard_normal((16, 1, 3, 3), dtype=np.float32) * 0.1,
        "conv1_b": np.zeros(16, np.float32),
        "conv2_w": rng.standard_normal((32, 16, 3, 3), dtype=np.float32) * 0.05,
        "conv2_b": np.zeros(32, np.float32),
        "values": np.sort(rng.standard_normal(4096).astype(np.float32) * 0.01),
        "w_idx1": rng.integers(0, 4096, (512, FEAT), dtype=np.int32),
        "fc1_b": np.zeros(512, np.float32),
        "w_idx2": rng.integers(0, 4096, (4, 512), dtype=np.int32),
        "fc2_b": np.zeros(4, np.float32),
    }
    out = kernel(**ins)
    print("out shape", out.shape, "sample row", out[0])


# revision 30
# speedup vs baseline: 1.1746x; 1.1746x over previous
"""Trainium2 Bass kernel for nn_MemristorCNN (embedding_lookup, 8 cores).

Strategy (per sharding hint):
- Host gathers the codebook weight W1 = values[w_idx1] and ships the
  *gathered weight* in bf16, column-sharded over in_features (12544
  features = 4 conv2 output channels per core), laid out [128, 99, 512]
  so the whole 12.9 MB stream is ONE DMA instruction (issue-rate, not
  bandwidth, limited the old 98-DMA stream); tile k=98 carries
  fc1_bias/8 against an on-device ones-column so the bias rides the
  PSUM accumulation and survives the ReduceScatter sum.
- Conv stack runs data-parallel (4 images per core); conv1 packs
  (tap, half-image) into K=72 with dx pre-shifted on host; conv2 packs
  (image, channel, dx-pair) into K=128 with 6 tap passes.
- AllToAll redistributes conv output h from batch-sharded to
  feature-sharded with an img-major payload so the receive buffer is a
  contiguous [32, 12544] view; ONE xbar transpose-DMA (out[p,k,i] =
  in[i, 128k+p]) lands it feature-major in SBUF, replacing 98 PE
  transposes; fc1 accumulates 99 matmuls; ReduceScatter sums partials;
  relu + fc2 (DVE mult+reduce) finish on device and the host
  concatenates the per-core [4, 4] outputs.
"""

import sys

import numpy as np
import ml_dtypes

BF16NP = ml_dtypes.bfloat16

for _p in ("/opt/trn_rl_repo",):
    if _p not in sys.path:
        sys.path.insert(0, _p)

import concourse.bacc as bacc
import concourse.bass as bass  # noqa: F401
import concourse.tile as tile
from concourse import mybir
from concourse.bass_utils import run_bass_kernel_spmd

F32 = mybir.dt.float32
BF16 = mybir.dt.bfloat16
RELU = mybir.ActivationFunctionType.Relu
COPY = mybir.ActivationFunctionType.Copy

N_CORES = 8
B = 32
IMG = 224
C1, C2 = 16, 32
PH, PW = 112, 112
HH, HW = 56, 56
FEAT = C2 * HH * HW          # 100352
FSH = FEAT // N_CORES        # 12544
NK = FSH // 128              # 98
NKB = NK + 1                 # +1 bias tile
H1 = 512
NOUT = 4

_CACHE = {}


def _build_program(stop_after: str = 'full'):
    nc = bacc.Bacc("TRN2", target_bir_lowering=False, debug=False,
                   num_devices=N_CORES)
    _emit(nc, stop_after)
    nc.compile()
    return nc


def _emit(nc, stop_after: str):
    # ---- kernel I/O ----
    x9_t = nc.dram_tensor("x9", [72, PH, 232], BF16, kind="ExternalInput")
    s1_t = nc.dram_tensor("s1", [72, 128], BF16, kind="ExternalInput")
    s2_t = nc.dram_tensor("s2", [6, 128, 128], BF16, kind="ExternalInput")
    w1t_t = nc.dram_tensor("w1t", [128, NKB * H1], BF16, kind="ExternalInput")
    w2r_t = nc.dram_tensor("w2r", [4, NOUT, H1], F32, kind="ExternalInput")
    b2t_t = nc.dram_tensor("b2t", [4, 4], F32, kind="ExternalInput")
    cb1_t = nc.dram_tensor("cb1", [128, 1], F32, kind="ExternalInput")
    cb2_t = nc.dram_tensor("cb2", [128, 1], F32, kind="ExternalInput")
    if stop_after in ("dumphT", "dumpw", "dumpfc1"):
        shp = {"dumphT": [128, NKB, 32], "dumpw": [128, NKB, H1],
               "dumpfc1": [B, H1]}[stop_after]
        out_t = nc.dram_tensor("out", shp, F32, kind="ExternalOutput")
    elif stop_after in ("dumpa2ain", "dumpa2aout"):
        out_t = nc.dram_tensor("out", [B, FSH], BF16, kind="ExternalOutput")
    elif stop_after == "dumph":
        out_t = nc.dram_tensor("out", [128, 3136], BF16, kind="ExternalOutput")
    else:
        out_t = nc.dram_tensor("out", [4, NOUT], F32, kind="ExternalOutput")

    # ---- internal DRAM (collective bounce buffers) ----
    a2a_in = nc.dram_tensor("a2a_in", [B, FSH], BF16)
    a2a_out = nc.dram_tensor("a2a_out", [N_CORES, 4, FSH], BF16)
    rs_in = nc.dram_tensor("rs_in", [B, H1], F32)
    rs_out = nc.dram_tensor("rs_out", [4, H1], F32)

    groups = [list(range(N_CORES))]

    with tile.TileContext(nc) as tc:
        with (
            tc.tile_pool(name="const", bufs=1) as cpool,
            tc.tile_pool(name="ps", bufs=4, space="PSUM") as pspool,
            tc.tile_pool(name="work", bufs=2) as wkpool,
            tc.tile_pool(name="xin", bufs=2) as xpool,
            tc.tile_pool(name="persist", bufs=1) as pers,
        ):
            # -------- latency-critical loads first --------
            # conv1 input: partition (dy*3+dx)*8 + h holds
            # x_pad[img(h), y0(h)+dy+y, dx+c]; 14 8-row chunks,
            # triple-buffered.
            NXC = 14
            x9_tiles = []
            for e in range(NXC):
                x9e = xpool.tile([72, 8, 232], BF16, tag="x9")
                if e == 0:
                    nc.sync.dma_start(out=x9e[:, :, :],
                                      in_=x9_t[:, 0:8, :])
                x9_tiles.append(x9e)
            s1_sb = cpool.tile([72, 128], BF16, tag="s1")
            nc.sync.dma_start(out=s1_sb[:, :], in_=s1_t[:, :])
            for e in range(1, NXC):
                nc.sync.dma_start(out=x9_tiles[e][:, :, :],
                                  in_=x9_t[:, 8 * e:8 * e + 8, :])

            # small constants first on the scalar ring (FIFO per ring —
            # anything queued after the big w1t DMA drains after it)
            cb1_sb = cpool.tile([128, 1], F32, tag="cb1")
            nc.scalar.dma_start(out=cb1_sb[:, :], in_=cb1_t[:, :])
            cb2_sb = cpool.tile([128, 1], F32, tag="cb2")
            nc.scalar.dma_start(out=cb2_sb[:, :], in_=cb2_t[:, :])
            s2_sb = cpool.tile([128, 6, 128], BF16, tag="s2")
            nc.scalar.dma_start(out=s2_sb[:, :, :],
                                in_=s2_t[:, :, :].rearrange("t p m -> p t m"))

            w2r_sb = cpool.tile([4, NOUT, H1], F32, tag="w2r")
            nc.scalar.dma_start(out=w2r_sb[:, :, :], in_=w2r_t[:, :, :])
            b2t_sb = cpool.tile([4, 4], F32, tag="b2t")
            nc.scalar.dma_start(out=b2t_sb[:, :], in_=b2t_t[:, :])

            # fc1 weight stream: 13 chunked DMAs queued at the TAIL of the
            # sync ring, behind the x9 loads.  Ring FIFO + the x9
            # buffer-reuse waits mean the weight bytes only start draining
            # once conv1 is well underway — the latency-critical conv
            # traffic is not starved by the 12.9 MB stream (weights are
            # only needed by fc1, ~100 us in).
            WCH = 8                         # k-tiles per chunk
            wch_edges = list(range(0, NKB, WCH)) + [NKB]
            if stop_after not in ("dumpa2ain", "dumpa2aout"):
                wts = pers.tile([128, NKB, H1], BF16, tag="w1")
                wflat = wts[:, :, :].rearrange("p k m -> p (k m)")
                for k0, k1 in zip(wch_edges[:-1], wch_edges[1:]):
                    nc.sync.dma_start(out=wflat[:, H1 * k0:H1 * k1],
                                      in_=w1t_t[:, H1 * k0:H1 * k1])

            # conv2 input buffer: partition e*64 + img*16 + ch holds the
            # padded channel image, dx-shifted by e.  Repack fills rows
            # 1..112 full-width; only rows 0/113 need zeroing.
            c2in = pers.tile([128, 114, 117], BF16, tag="bigC")
            nc.gpsimd.memset(c2in[:, 0, :], 0.0)
            nc.gpsimd.memset(c2in[:, 113, :], 0.0)

            # pool1 rows are 117 wide with zeroed borders (cols 0,
            # 113-116 + one spare element) so the repack shifts by e via
            # a single contiguous flat copy per (chunk, half, e).
            pool1_a = pers.tile([128, 28 * 117 + 1], BF16, tag="bigB1")
            pool1_b = pers.tile([128, 28 * 117 + 1], BF16, tag="bigB2")
            pool1_parts = [pool1_a, pool1_b]
            pool1_views = []
            for t in pool1_parts:
                pv = t[:, 0:28 * 117].rearrange("p (r c) -> p r c", c=117)
                nc.gpsimd.memset(pv[:, :, 0], 0.0)
                nc.gpsimd.memset(pv[:, :, 113:117], 0.0)
                nc.gpsimd.memset(t[:, 28 * 117:], 0.0)
                pool1_views.append(pv)

            # fc1 bias rides k-tile 98: ones column on partition 0
            hT = pers.tile([128, NKB, 32], BF16, tag="bigHT")
            nc.gpsimd.memset(hT[:, NK, :], 0.0)
            nc.gpsimd.memset(hT[0:1, NK, :], 1.0)

            # ---------------- conv1 + pool1 + relu ----------------
            # out partition m = h*16 + oc = half*64 + img*16 + oc
            for T in range(28):            # 2 pooled rows per psum tile
                ps = pspool.tile([128, 2, 512], F32, tag="ps")
                for g in range(2):
                    yp = T * 2 + g         # pooled row within half
                    e, ypl = yp // 4, yp % 4
                    nc.tensor.matmul(
                        ps[:, g, 0:448],
                        lhsT=s1_sb[:, :],
                        rhs=x9_tiles[e][:, 2 * ypl:2 * ypl + 2, :224],
                        start=True, stop=True)
                v = ps[:, :, 0:448].rearrange("p g (r x w) -> p g r x w",
                                              r=2, w=2)
                c1 = wkpool.tile([128, 2, 2, 112], F32, tag="mc")
                nc.scalar.activation(c1[:, :, :, :], v[:, :, :, :, 1], COPY)
                m1 = wkpool.tile([128, 2, 2, 112], F32, tag="mx")
                nc.vector.tensor_max(m1[:, :, :, :], v[:, :, :, :, 0],
                                     c1[:, :, :, :])
                m2 = wkpool.tile([128, 2, 112], F32, tag="mxb")
                nc.vector.tensor_max(m2[:, :, :], m1[:, :, 0, :],
                                     m1[:, :, 1, :])
                half_t, row_t = divmod(2 * T, 28)
                nc.scalar.activation(
                    pool1_views[half_t][:, row_t:row_t + 2, 1:113],
                    m2[:, :, :], RELU, bias=cb1_sb[:, :])

            if stop_after == "conv1":
                dbg = wkpool.tile([4, NOUT], F32, tag="outsb")
                nc.vector.tensor_copy(dbg[:, :], pool1_views[0][0:4, 0, 1:5])
                nc.sync.dma_start(out=out_t[:, :], in_=dbg[:, :])
                return

            # -------- repack pool1 -> conv2 input (padded, merged halves,
            # two dx-shifted copies); 8 flat contiguous DMAs (one
            # descriptor per partition).  c2in[R0+r, c] = pool[r, c+e];
            # the pool's zeroed borders supply the conv padding.
            c2flat = c2in[:, :, :].rearrange("p r c -> p (r c)")
            for chunk in range(2):
                for half in range(2):
                    r0 = 117 * (56 * half + 1 + 28 * chunk)
                    for e in range(2):
                        nc.scalar.dma_start(
                            out=c2flat[64 * e:64 * e + 64,
                                       r0:r0 + 28 * 117],
                            in_=pool1_parts[chunk][64 * half:64 * half + 64,
                                                   e:e + 28 * 117])

            # ---------------- conv2 + pool2 + relu ----------------
            # out partition m = img*32 + oc; 6 passes t=(dy, grp):
            # partition block e supplies tap dx = 2*grp + e.
            h_sb = pers.tile([128, 7, 4, 2, 56], BF16, tag="bigD")
            for T in range(14):            # 8 conv rows / 4 pooled rows
                ps = pspool.tile([128, 2, 512], F32, tag="ps")
                for sub in range(2):
                    y0 = 8 * T + 4 * sub
                    for t in range(6):
                        dy, grp = t // 2, t % 2
                        nc.tensor.matmul(
                            ps[:, sub, 0:448],
                            lhsT=s2_sb[:, t, :],
                            rhs=c2in[:, y0 + dy:y0 + dy + 4,
                                     2 * grp:2 * grp + 112],
                            start=(t == 0), stop=(t == 5))
                v = ps[:, :, 0:448].rearrange("p s (r x w) -> p s r x w",
                                              r=4, w=2)
                c1 = wkpool.tile([128, 2, 4, 56], F32, tag="mc2")
                nc.scalar.activation(c1[:, :, :, :], v[:, :, :, :, 1], COPY)
                m1 = wkpool.tile([128, 2, 4, 56], F32, tag="mx2")
                nc.vector.tensor_max(m1[:, :, :, :], v[:, :, :, :, 0],
                                     c1[:, :, :, :])
                v2 = m1[:, :, :, :].rearrange("p s (rp w) x -> p s rp w x",
                                              w=2)
                m2 = wkpool.tile([128, 2, 2, 56], F32, tag="mxb2")
                nc.vector.tensor_max(m2[:, :, :, :], v2[:, :, :, 0, :],
                                     v2[:, :, :, 1, :])
                # pooled rows 4T..4T+4 -> h_sb[T//2, 2*(T%2) + (0..1), ...]
                nc.scalar.activation(
                    h_sb[:, T // 2, 2 * (T % 2):2 * (T % 2) + 2, :, :],
                    m2[:, :, :, :], RELU, bias=cb2_sb[:, :])

            if stop_after == "conv2":
                dbg = wkpool.tile([4, NOUT], F32, tag="outsb")
                nc.vector.tensor_copy(dbg[:, :], h_sb[0:4, 0, 0, 0, 0:4])
                nc.sync.dma_start(out=out_t[:, :], in_=dbg[:, :])
                return

            # -------- AllToAll: batch-shard -> feature-shard --------
            # payload row (4*dest + img_loc), col (ch_loc, sp): receiver
            # gets a contiguous [32, 12544] img-major view.  One DMA per
            # local image (DMA APs are limited to 3 dims).
            h_flat = h_sb[:, :, :, :, :].rearrange(
                "(i m) t j r x -> i m (t j r x)", i=4)
            for il in range(4):
                nc.scalar.dma_start(
                    out=a2a_in[:, :].rearrange(
                        "(d i) (c s) -> i d c s", d=8, c=4)[il],
                    in_=h_flat[il])
            if stop_after == "dumph":
                nc.sync.dma_start(
                    out=out_t[:, :],
                    in_=h_sb[:, :, :, :, :].rearrange("p t j r x -> p (t j r x)"))
                return

            if stop_after == "dumpa2ain":
                dbg = pers.tile([32, FSH], BF16, tag="dmpa")
                nc.sync.dma_start(out=dbg[:, :], in_=a2a_in[:, :])
                nc.sync.dma_start(out=out_t[:, :], in_=dbg[:, :])
                return

            nc.gpsimd.collective_compute(
                "AllToAll", mybir.AluOpType.bypass, replica_groups=groups,
                ins=[a2a_in[:, :]], outs=[a2a_out[:, :, :]])

            if stop_after == "dumpa2aout":
                dbg = pers.tile([32, FSH], BF16, tag="dmpa")
                nc.sync.dma_start(
                    out=dbg[:, :],
                    in_=a2a_out[:, :, :].rearrange("s i f -> (s i) f"))
                nc.sync.dma_start(out=out_t[:, :], in_=dbg[:, :])
                return

            # -------- ONE xbar transpose-DMA -> feature-major hT --------
            # out[p, k, i] = in[i, 128k + p]
            nc.scalar.dma_start(
                out=hT[:, 0:NK, :],
                in_=a2a_out[:, :, :].rearrange("s i f -> (s i) f"),
                transpose=True)

            if stop_after == "a2a":
                dbg = wkpool.tile([4, NOUT], F32, tag="outsb")
                nc.vector.tensor_copy(dbg[:, :], hT[0:4, 0, 0:4])
                nc.sync.dma_start(out=out_t[:, :], in_=dbg[:, :])
                return

            if stop_after in ("dumphT", "dumpw"):
                src = hT if stop_after == "dumphT" else wts
                n = 32 if stop_after == "dumphT" else H1
                for k in range(NKB):
                    dbg = wkpool.tile([128, n], F32, tag="dmp")
                    nc.vector.tensor_copy(dbg[:, :], src[:, k, :])
                    nc.sync.dma_start(out=out_t[:, k, :], in_=dbg[:, :])
                return

            # ---------------- fc1 partial ----------------
            fc1_ps = pspool.tile([32, H1], F32, tag="ps")
            for k in range(NKB):
                nc.tensor.matmul(fc1_ps[:, :], lhsT=hT[:, k, :],
                                 rhs=wts[:, k, :],
                                 start=(k == 0), stop=(k == NKB - 1))
            fc1_sb = wkpool.tile([B, H1], F32, tag="fc1")
            nc.vector.tensor_copy(fc1_sb[:, :], fc1_ps[:, :])
            nc.scalar.dma_start(out=rs_in[:, :], in_=fc1_sb[:, :])

            if stop_after == "fc1":
                nc.sync.dma_start(out=out_t[:, :], in_=fc1_sb[0:4, 0:4])
                return

            # -------- ReduceScatter + relu + fc2 --------
            nc.gpsimd.collective_compute(
                "ReduceScatter", mybir.AluOpType.add, replica_groups=groups,
                ins=[rs_in[:, :]], outs=[rs_out[:, :]])

            h1 = wkpool.tile([4, H1], F32, tag="h1")
            nc.scalar.dma_start(out=h1[:, :], in_=rs_out[:, :])
            nc.scalar.activation(h1[:, :], h1[:, :], RELU)

            if stop_after == "rs":
                dbg = wkpool.tile([4, NOUT], F32, tag="outsb")
                nc.vector.tensor_copy(dbg[:, :], h1[0:4, 0:4])
                nc.sync.dma_start(out=out_t[:, :], in_=dbg[:, :])
                return

            out_sb = wkpool.tile([4, NOUT], F32, tag="outsb")
            for o in range(NOUT):
                prod = wkpool.tile([4, H1], F32, tag="prod")
                nc.vector.tensor_mul(prod[:, :], h1[:, :], w2r_sb[:, o, :])
                nc.vector.tensor_reduce(
                    out_sb[:, o:o + 1], prod[:, :],
                    axis=mybir.AxisListType.X, op=mybir.AluOpType.add)
            nc.vector.tensor_add(out_sb[:, :], out_sb[:, :], b2t_sb[:, :])
            nc.scalar.dma_start(out=out_t[:, :], in_=out_sb[:, :])


def _get_program():
    key = ("prog",)
    if key not in _CACHE:
        _CACHE[key] = _build_program()
    return _CACHE[key]


def _host_prep(x, conv1_w, conv1_b, conv2_w, conv2_b, values, w_idx1,
               fc1_b, w_idx2, fc2_b):
    """Build per-core input maps (numpy, bf16 for PE-facing tensors)."""
    f32 = np.float32
    x = np.asarray(x, f32)
    conv1_w = np.asarray(conv1_w, f32)
    conv2_w = np.asarray(conv2_w, f32)
    values = np.asarray(values, f32)
    w_idx1 = np.asarray(w_idx1)
    w_idx2 = np.asarray(w_idx2)

    x_pad = np.zeros((B, 226, 232), f32)
    x_pad[:, 1:225, 1:225] = x[:, 0]

    # x9[c]: [72, 112, 232]; partition (dy*3+dx)*8 + h, h = 4*half + img_loc
    x9 = np.zeros((N_CORES, 72, PH, 232), f32)
    for dy in range(3):
        for dx in range(3):
            for h in range(8):
                half, il = h // 4, h % 4
                y0 = PH * half
                for c in range(N_CORES):
                    x9[c, (dy * 3 + dx) * 8 + h, :, :232 - dx] = \
                        x_pad[4 * c + il, y0 + dy:y0 + dy + PH, dx:]

    s1 = np.zeros((72, 128), f32)
    for dy in range(3):
        for dx in range(3):
            for h in range(8):
                s1[(dy * 3 + dx) * 8 + h, 16 * h:16 * h + C1] = \
                    conv1_w[:, 0, dy, dx]

    # conv2 stationaries [6, 128, 128]: pass t = dy*2 + grp;
    # partition p = e*64 + img*16 + ch supplies tap dx = 2*grp + e
    s2 = np.zeros((6, 128, 128), f32)
    for t in range(6):
        dy, grp = t // 2, t % 2
        for e in range(2):
            dx = 2 * grp + e
            if dx > 2:
                continue
            for img in range(4):
                for ch in range(C1):
                    s2[t, 64 * e + 16 * img + ch, 32 * img:32 * img + C2] = \
                        conv2_w[:, ch, dy, dx]

    # fc1 weight [128, 99, 512]: tile k row p = feature 128k+p of the
    # core's shard; tile 98 row 0 = fc1_b/8 (ones-column bias trick).
    b1_8 = np.asarray(fc1_b, f32) / N_CORES
    w1ts = []
    for c in range(N_CORES):
        idx = w_idx1[:, FSH * c:FSH * (c + 1)]             # [512, 12544]
        wt = np.zeros((128, NKB, H1), f32)
        wt[:, :NK, :] = values[idx].T.reshape(NK, 128, H1).transpose(1, 0, 2)
        wt[0, NK, :] = b1_8
        w1ts.append(np.ascontiguousarray(
            wt.reshape(128, NKB * H1)).astype(BF16NP))

    w2 = np.ascontiguousarray(values[w_idx2]).astype(f32)     # [4, 512]
    w2r = np.broadcast_to(w2[None, :, :], (4, NOUT, H1)).copy()
    b2t = np.broadcast_to(np.asarray(fc2_b, f32), (4, 4)).copy()

    cb1 = np.zeros((128, 1), f32)
    for h in range(8):
        cb1[16 * h:16 * h + C1, 0] = np.asarray(conv1_b, f32)
    cb2 = np.zeros((128, 1), f32)
    for img in range(4):
        cb2[32 * img:32 * img + C2, 0] = np.asarray(conv2_b, f32)

    s1 = s1.astype(BF16NP)
    s2 = s2.astype(BF16NP)
    in_maps = []
    for c in range(N_CORES):
        in_maps.append({
            "x9": np.ascontiguousarray(x9[c]).astype(BF16NP),
            "s1": s1, "s2": s2,
            "w1t": w1ts[c],
            "w2r": w2r, "b2t": b2t,
            "cb1": cb1, "cb2": cb2,
        })
    return in_maps


def kernel(x, conv1_w, conv1_b, conv2_w, conv2_b, values, w_idx1, fc1_b,
           w_idx2, fc2_b, _trace=False, _trace_kwargs=None):
    nc = _get_program()
    in_maps = _host_prep(x, conv1_w, conv1_b, conv2_w, conv2_b, values,
                         w_idx1, fc1_b, w_idx2, fc2_b)
    res = run_bass_kernel_spmd(nc, in_maps, core_ids=list(range(N_CORES)),
                               trace=_trace, **(_trace_kwargs or {}))
    out = np.zeros((B, NOUT), np.float32)
    for c in range(N_CORES):
        out[4 * c:4 * c + 4] = res.results[c]["out"]
    if _trace:
        kernel.last_result = res
    return out


if __name__ == "__main__":
    rng = np.random.default_rng(0)
    ins = {
        "x": rng.standard_normal((B, 1, IMG, IMG), dtype=np.float32),
        "conv1_w": rng.standard_normal((16, 1, 3, 3), dtype=np.float32) * 0.1,
        "conv1_b": np.zeros(16, np.float32),
        "conv2_w": rng.standard_normal((32, 16, 3, 3), dtype=np.float32) * 0.05,
        "conv2_b": np.zeros(32, np.float32),
        "values": np.sort(rng.standard_normal(4096).astype(np.float32) * 0.01),
        "w_idx1": rng.integers(0, 4096, (512, FEAT), dtype=np.int32),
        "fc1_b": np.zeros(512, np.float32),
        "w_idx2": rng.integers(0, 4096, (4, 512), dtype=np.int32),
        "fc2_b": np.zeros(4, np.float32),
    }
    out = kernel(**ins)
    print("out shape", out.shape, "sample row", out[0])


# revision 33
# speedup vs baseline: 1.2611x; 1.0736x over previous
"""Trainium2 Bass kernel for nn_MemristorCNN (embedding_lookup, 8 cores).

Strategy (per sharding hint):
- Host gathers the codebook weight W1 = values[w_idx1] and ships the
  *gathered weight* in bf16, column-sharded over in_features (12544
  features = 4 conv2 output channels per core), laid out [128, 99, 512]
  so the whole 12.9 MB stream is ONE DMA instruction (issue-rate, not
  bandwidth, limited the old 98-DMA stream); tile k=98 carries
  fc1_bias/8 against an on-device ones-column so the bias rides the
  PSUM accumulation and survives the ReduceScatter sum.
- Conv stack runs data-parallel (4 images per core); conv1 packs
  (tap, half-image) into K=72 with dx pre-shifted on host; conv2 packs
  (image, channel, dx-pair) into K=128 with 6 tap passes.
- AllToAll redistributes conv output h from batch-sharded to
  feature-sharded with an img-major payload so the receive buffer is a
  contiguous [32, 12544] view; ONE xbar transpose-DMA (out[p,k,i] =
  in[i, 128k+p]) lands it feature-major in SBUF, replacing 98 PE
  transposes; fc1 accumulates 99 matmuls; ReduceScatter sums partials;
  relu + fc2 (DVE mult+reduce) finish on device and the host
  concatenates the per-core [4, 4] outputs.
"""

import sys

import numpy as np
import ml_dtypes

BF16NP = ml_dtypes.bfloat16

for _p in ("/opt/trn_rl_repo",):
    if _p not in sys.path:
        sys.path.insert(0, _p)

import concourse.bacc as bacc
import concourse.bass as bass  # noqa: F401
import concourse.tile as tile
from concourse import mybir
from concourse.bass_utils import run_bass_kernel_spmd

F32 = mybir.dt.float32
BF16 = mybir.dt.bfloat16
RELU = mybir.ActivationFunctionType.Relu
COPY = mybir.ActivationFunctionType.Copy

N_CORES = 8
B = 32
IMG = 224
C1, C2 = 16, 32
PH, PW = 112, 112
HH, HW = 56, 56
FEAT = C2 * HH * HW          # 100352
FSH = FEAT // N_CORES        # 12544
NK = FSH // 128              # 98
NKB = NK + 1                 # +1 bias tile
H1 = 512
NOUT = 4

_CACHE = {}


def _build_program(stop_after: str = 'full'):
    nc = bacc.Bacc("TRN2", target_bir_lowering=False, debug=False,
                   num_devices=N_CORES)
    _emit(nc, stop_after)
    nc.compile()
    return nc


def _emit(nc, stop_after: str):
    # ---- kernel I/O ----
    x9_t = nc.dram_tensor("x9", [72, PH, 232], BF16, kind="ExternalInput")
    s1_t = nc.dram_tensor("s1", [72, 128], BF16, kind="ExternalInput")
    s2_t = nc.dram_tensor("s2", [6, 128, 128], BF16, kind="ExternalInput")
    w1t_t = nc.dram_tensor("w1t", [128, NKB * H1], BF16, kind="ExternalInput")
    w2r_t = nc.dram_tensor("w2r", [4, NOUT, H1], F32, kind="ExternalInput")
    b2t_t = nc.dram_tensor("b2t", [4, 4], F32, kind="ExternalInput")
    cb1_t = nc.dram_tensor("cb1", [128, 1], F32, kind="ExternalInput")
    cb2_t = nc.dram_tensor("cb2", [128, 1], F32, kind="ExternalInput")
    if stop_after in ("dumphT", "dumpw", "dumpfc1"):
        shp = {"dumphT": [128, NKB, 32], "dumpw": [128, NKB, H1],
               "dumpfc1": [B, H1]}[stop_after]
        out_t = nc.dram_tensor("out", shp, F32, kind="ExternalOutput")
    elif stop_after in ("dumpa2ain", "dumpa2aout"):
        out_t = nc.dram_tensor("out", [B, FSH], BF16, kind="ExternalOutput")
    elif stop_after == "dumph":
        out_t = nc.dram_tensor("out", [128, 3136], BF16, kind="ExternalOutput")
    else:
        out_t = nc.dram_tensor("out", [4, NOUT], F32, kind="ExternalOutput")

    # ---- internal DRAM (collective bounce buffers) ----
    a2a_in = nc.dram_tensor("a2a_in", [B, FSH], BF16)
    a2a_out = nc.dram_tensor("a2a_out", [N_CORES, 4, FSH], BF16)
    rs_in = nc.dram_tensor("rs_in", [B, H1], F32)
    rs_out = nc.dram_tensor("rs_out", [4, H1], F32)

    groups = [list(range(N_CORES))]

    with tile.TileContext(nc) as tc:
        with (
            tc.tile_pool(name="const", bufs=1) as cpool,
            tc.tile_pool(name="ps", bufs=4, space="PSUM") as pspool,
            tc.tile_pool(name="work", bufs=2) as wkpool,
            tc.tile_pool(name="xin", bufs=2) as xpool,
            tc.tile_pool(name="persist", bufs=1) as pers,
        ):
            # -------- latency-critical loads first --------
            # conv1 input: partition (dy*3+dx)*8 + h holds
            # x_pad[img(h), y0(h)+dy+y, dx+c]; 14 8-row chunks,
            # triple-buffered.
            NXC = 14
            x9_tiles = []
            for e in range(NXC):
                x9e = xpool.tile([72, 8, 232], BF16, tag="x9")
                if e == 0:
                    nc.sync.dma_start(out=x9e[:, :, :],
                                      in_=x9_t[:, 0:8, :])
                x9_tiles.append(x9e)
            s1_sb = cpool.tile([72, 128], BF16, tag="s1")
            nc.sync.dma_start(out=s1_sb[:, :], in_=s1_t[:, :])
            for e in range(1, NXC):
                nc.sync.dma_start(out=x9_tiles[e][:, :, :],
                                  in_=x9_t[:, 8 * e:8 * e + 8, :])

            # small constants first on the scalar ring (FIFO per ring —
            # anything queued after the big w1t DMA drains after it)
            cb1_sb = cpool.tile([128, 1], F32, tag="cb1")
            nc.scalar.dma_start(out=cb1_sb[:, :], in_=cb1_t[:, :])
            cb2_sb = cpool.tile([128, 1], F32, tag="cb2")
            nc.scalar.dma_start(out=cb2_sb[:, :], in_=cb2_t[:, :])
            s2_sb = cpool.tile([128, 6, 128], BF16, tag="s2")
            nc.scalar.dma_start(out=s2_sb[:, :, :],
                                in_=s2_t[:, :, :].rearrange("t p m -> p t m"))

            w2r_sb = cpool.tile([4, NOUT, H1], F32, tag="w2r")
            nc.scalar.dma_start(out=w2r_sb[:, :, :], in_=w2r_t[:, :, :])
            b2t_sb = cpool.tile([4, 4], F32, tag="b2t")
            nc.scalar.dma_start(out=b2t_sb[:, :], in_=b2t_t[:, :])

            # fc1 weight stream: 13 chunked DMAs, each gated on conv
            # progress via a 1-element ACT write into its destination
            # (WAW dep) — weights (needed only by fc1, ~100 us in) drain
            # paced by conv instead of starving the conv-critical
            # x9/repack traffic.  Chunk DMAs are emitted inside the conv
            # loops; see _wchunk().
            WCH = 8                         # k-tiles per chunk
            wch_edges = list(range(0, NKB, WCH)) + [NKB]
            have_w = stop_after not in ("dumpa2ain", "dumpa2aout")
            if have_w:
                wts = pers.tile([128, NKB, H1], BF16, tag="w1")
                wflat = wts[:, :, :].rearrange("p k m -> p (k m)")

            def _wchunk(i, gate_src):
                if not have_w or i >= len(wch_edges) - 1:
                    return
                k0, k1 = wch_edges[i], wch_edges[i + 1]
                nc.scalar.activation(wts[0:1, k0, 0:1], gate_src, COPY)
                nc.sync.dma_start(out=wflat[:, H1 * k0:H1 * k1],
                                  in_=w1t_t[:, H1 * k0:H1 * k1])

            # conv2 input buffer: partition e*64 + img*16 + ch holds the
            # padded channel image, dx-shifted by e.  Repack fills rows
            # 1..112 full-width; only rows 0/113 need zeroing.
            c2in = pers.tile([128, 114, 117], BF16, tag="bigC")
            nc.gpsimd.memset(c2in[:, 0, :], 0.0)
            nc.gpsimd.memset(c2in[:, 113, :], 0.0)

            # pool1 rows are 117 wide with zeroed borders (cols 0,
            # 113-116 + one spare element) so the repack shifts by e via
            # a single contiguous flat copy per (chunk, half, e).
            pool1_a = pers.tile([128, 28 * 117 + 1], BF16, tag="bigB1")
            pool1_b = pers.tile([128, 28 * 117 + 1], BF16, tag="bigB2")
            pool1_parts = [pool1_a, pool1_b]
            pool1_views = []
            for t in pool1_parts:
                pv = t[:, 0:28 * 117].rearrange("p (r c) -> p r c", c=117)
                nc.gpsimd.memset(pv[:, :, 0], 0.0)
                nc.gpsimd.memset(pv[:, :, 113:117], 0.0)
                nc.gpsimd.memset(t[:, 28 * 117:], 0.0)
                pool1_views.append(pv)

            # fc1 bias rides k-tile 98: ones column on partition 0
            hT = pers.tile([128, NKB, 32], BF16, tag="bigHT")
            nc.gpsimd.memset(hT[:, NK, :], 0.0)
            nc.gpsimd.memset(hT[0:1, NK, :], 1.0)

            # ---------------- conv1 + pool1 + relu ----------------
            # out partition m = h*16 + oc = half*64 + img*16 + oc
            for T in range(28):            # 2 pooled rows per psum tile
                ps = pspool.tile([128, 2, 512], F32, tag="ps")
                for g in range(2):
                    yp = T * 2 + g         # pooled row within half
                    e, ypl = yp // 4, yp % 4
                    nc.tensor.matmul(
                        ps[:, g, 0:448],
                        lhsT=s1_sb[:, :],
                        rhs=x9_tiles[e][:, 2 * ypl:2 * ypl + 2, :224],
                        start=True, stop=True)
                v = ps[:, :, 0:448].rearrange("p g (r x w) -> p g r x w",
                                              r=2, w=2)
                c1 = wkpool.tile([128, 2, 2, 112], F32, tag="mc")
                nc.scalar.activation(c1[:, :, :, :], v[:, :, :, :, 1], COPY)
                m1 = wkpool.tile([128, 2, 2, 112], F32, tag="mx")
                nc.vector.tensor_max(m1[:, :, :, :], v[:, :, :, :, 0],
                                     c1[:, :, :, :])
                m2 = wkpool.tile([128, 2, 112], F32, tag="mxb")
                nc.vector.tensor_max(m2[:, :, :], m1[:, :, 0, :],
                                     m1[:, :, 1, :])
                half_t, row_t = divmod(2 * T, 28)
                nc.scalar.activation(
                    pool1_views[half_t][:, row_t:row_t + 2, 1:113],
                    m2[:, :, :], RELU, bias=cb1_sb[:, :])
                # first 3 weight chunks gated on late conv1 progress
                if T in (10, 18, 26):
                    _wchunk((T - 10) // 8,
                            pool1_views[half_t][0:1, row_t, 1:2])

            if stop_after == "conv1":
                dbg = wkpool.tile([4, NOUT], F32, tag="outsb")
                nc.vector.tensor_copy(dbg[:, :], pool1_views[0][0:4, 0, 1:5])
                nc.sync.dma_start(out=out_t[:, :], in_=dbg[:, :])
                return

            # -------- repack pool1 -> conv2 input (padded, merged halves,
            # two dx-shifted copies); 8 flat contiguous DMAs (one
            # descriptor per partition).  c2in[R0+r, c] = pool[r, c+e];
            # the pool's zeroed borders supply the conv padding.
            c2flat = c2in[:, :, :].rearrange("p r c -> p (r c)")
            for chunk in range(2):
                for half in range(2):
                    r0 = 117 * (56 * half + 1 + 28 * chunk)
                    for e in range(2):
                        nc.scalar.dma_start(
                            out=c2flat[64 * e:64 * e + 64,
                                       r0:r0 + 28 * 117],
                            in_=pool1_parts[chunk][64 * half:64 * half + 64,
                                                   e:e + 28 * 117])

            # ---------------- conv2 + pool2 + relu ----------------
            # out partition m = img*32 + oc; 6 passes t=(dy, grp):
            # partition block e supplies tap dx = 2*grp + e.
            h_sb = pers.tile([128, 7, 4, 2, 56], BF16, tag="bigD")
            for T in range(14):            # 8 conv rows / 4 pooled rows
                ps = pspool.tile([128, 2, 512], F32, tag="ps")
                for sub in range(2):
                    y0 = 8 * T + 4 * sub
                    for t in range(6):
                        dy, grp = t // 2, t % 2
                        nc.tensor.matmul(
                            ps[:, sub, 0:448],
                            lhsT=s2_sb[:, t, :],
                            rhs=c2in[:, y0 + dy:y0 + dy + 4,
                                     2 * grp:2 * grp + 112],
                            start=(t == 0), stop=(t == 5))
                v = ps[:, :, 0:448].rearrange("p s (r x w) -> p s r x w",
                                              r=4, w=2)
                c1 = wkpool.tile([128, 2, 4, 56], F32, tag="mc2")
                nc.scalar.activation(c1[:, :, :, :], v[:, :, :, :, 1], COPY)
                m1 = wkpool.tile([128, 2, 4, 56], F32, tag="mx2")
                nc.vector.tensor_max(m1[:, :, :, :], v[:, :, :, :, 0],
                                     c1[:, :, :, :])
                v2 = m1[:, :, :, :].rearrange("p s (rp w) x -> p s rp w x",
                                              w=2)
                m2 = wkpool.tile([128, 2, 2, 56], F32, tag="mxb2")
                nc.vector.tensor_max(m2[:, :, :, :], v2[:, :, :, 0, :],
                                     v2[:, :, :, 1, :])
                # pooled rows 4T..4T+4 -> h_sb[T//2, 2*(T%2) + (0..1), ...]
                nc.scalar.activation(
                    h_sb[:, T // 2, 2 * (T % 2):2 * (T % 2) + 2, :, :],
                    m2[:, :, :, :], RELU, bias=cb2_sb[:, :])
                # remaining weight chunks gated on conv2 progress
                _wchunk(3 + T, h_sb[0:1, T // 2, 2 * (T % 2), 0, 0:1])

            if stop_after == "conv2":
                dbg = wkpool.tile([4, NOUT], F32, tag="outsb")
                nc.vector.tensor_copy(dbg[:, :], h_sb[0:4, 0, 0, 0, 0:4])
                nc.sync.dma_start(out=out_t[:, :], in_=dbg[:, :])
                return

            # -------- AllToAll: batch-shard -> feature-shard --------
            # payload row (4*dest + img_loc), col (ch_loc, sp): receiver
            # gets a contiguous [32, 12544] img-major view.  One DMA per
            # local image (DMA APs are limited to 3 dims).
            h_flat = h_sb[:, :, :, :, :].rearrange(
                "(i m) t j r x -> i m (t j r x)", i=4)
            for il in range(4):
                nc.scalar.dma_start(
                    out=a2a_in[:, :].rearrange(
                        "(d i) (c s) -> i d c s", d=8, c=4)[il],
                    in_=h_flat[il])
            if stop_after == "dumph":
                nc.sync.dma_start(
                    out=out_t[:, :],
                    in_=h_sb[:, :, :, :, :].rearrange("p t j r x -> p (t j r x)"))
                return

            if stop_after == "dumpa2ain":
                dbg = pers.tile([32, FSH], BF16, tag="dmpa")
                nc.sync.dma_start(out=dbg[:, :], in_=a2a_in[:, :])
                nc.sync.dma_start(out=out_t[:, :], in_=dbg[:, :])
                return

            nc.gpsimd.collective_compute(
                "AllToAll", mybir.AluOpType.bypass, replica_groups=groups,
                ins=[a2a_in[:, :]], outs=[a2a_out[:, :, :]])

            if stop_after == "dumpa2aout":
                dbg = pers.tile([32, FSH], BF16, tag="dmpa")
                nc.sync.dma_start(
                    out=dbg[:, :],
                    in_=a2a_out[:, :, :].rearrange("s i f -> (s i) f"))
                nc.sync.dma_start(out=out_t[:, :], in_=dbg[:, :])
                return

            # -------- ONE xbar transpose-DMA -> feature-major hT --------
            # out[p, k, i] = in[i, 128k + p]
            nc.scalar.dma_start(
                out=hT[:, 0:NK, :],
                in_=a2a_out[:, :, :].rearrange("s i f -> (s i) f"),
                transpose=True)

            if stop_after == "a2a":
                dbg = wkpool.tile([4, NOUT], F32, tag="outsb")
                nc.vector.tensor_copy(dbg[:, :], hT[0:4, 0, 0:4])
                nc.sync.dma_start(out=out_t[:, :], in_=dbg[:, :])
                return

            if stop_after in ("dumphT", "dumpw"):
                src = hT if stop_after == "dumphT" else wts
                n = 32 if stop_after == "dumphT" else H1
                for k in range(NKB):
                    dbg = wkpool.tile([128, n], F32, tag="dmp")
                    nc.vector.tensor_copy(dbg[:, :], src[:, k, :])
                    nc.sync.dma_start(out=out_t[:, k, :], in_=dbg[:, :])
                return

            # ---------------- fc1 partial ----------------
            fc1_ps = pspool.tile([32, H1], F32, tag="ps")
            for k in range(NKB):
                nc.tensor.matmul(fc1_ps[:, :], lhsT=hT[:, k, :],
                                 rhs=wts[:, k, :],
                                 start=(k == 0), stop=(k == NKB - 1))
            fc1_sb = wkpool.tile([B, H1], F32, tag="fc1")
            nc.vector.tensor_copy(fc1_sb[:, :], fc1_ps[:, :])
            nc.scalar.dma_start(out=rs_in[:, :], in_=fc1_sb[:, :])

            if stop_after == "fc1":
                nc.sync.dma_start(out=out_t[:, :], in_=fc1_sb[0:4, 0:4])
                return

            # -------- ReduceScatter + relu + fc2 --------
            nc.gpsimd.collective_compute(
                "ReduceScatter", mybir.AluOpType.add, replica_groups=groups,
                ins=[rs_in[:, :]], outs=[rs_out[:, :]])

            h1 = wkpool.tile([4, H1], F32, tag="h1")
            nc.scalar.dma_start(out=h1[:, :], in_=rs_out[:, :])
            nc.scalar.activation(h1[:, :], h1[:, :], RELU)

            if stop_after == "rs":
                dbg = wkpool.tile([4, NOUT], F32, tag="outsb")
                nc.vector.tensor_copy(dbg[:, :], h1[0:4, 0:4])
                nc.sync.dma_start(out=out_t[:, :], in_=dbg[:, :])
                return

            out_sb = wkpool.tile([4, NOUT], F32, tag="outsb")
            for o in range(NOUT):
                prod = wkpool.tile([4, H1], F32, tag="prod")
                nc.vector.tensor_mul(prod[:, :], h1[:, :], w2r_sb[:, o, :])
                nc.vector.tensor_reduce(
                    out_sb[:, o:o + 1], prod[:, :],
                    axis=mybir.AxisListType.X, op=mybir.AluOpType.add)
            nc.vector.tensor_add(out_sb[:, :], out_sb[:, :], b2t_sb[:, :])
            nc.scalar.dma_start(out=out_t[:, :], in_=out_sb[:, :])


def _get_program():
    key = ("prog",)
    if key not in _CACHE:
        _CACHE[key] = _build_program()
    return _CACHE[key]


def _host_prep(x, conv1_w, conv1_b, conv2_w, conv2_b, values, w_idx1,
               fc1_b, w_idx2, fc2_b):
    """Build per-core input maps (numpy, bf16 for PE-facing tensors)."""
    f32 = np.float32
    x = np.asarray(x, f32)
    conv1_w = np.asarray(conv1_w, f32)
    conv2_w = np.asarray(conv2_w, f32)
    values = np.asarray(values, f32)
    w_idx1 = np.asarray(w_idx1)
    w_idx2 = np.asarray(w_idx2)

    x_pad = np.zeros((B, 226, 232), f32)
    x_pad[:, 1:225, 1:225] = x[:, 0]

    # x9[c]: [72, 112, 232]; partition (dy*3+dx)*8 + h, h = 4*half + img_loc
    x9 = np.zeros((N_CORES, 72, PH, 232), f32)
    for dy in range(3):
        for dx in range(3):
            for h in range(8):
                half, il = h // 4, h % 4
                y0 = PH * half
                for c in range(N_CORES):
                    x9[c, (dy * 3 + dx) * 8 + h, :, :232 - dx] = \
                        x_pad[4 * c + il, y0 + dy:y0 + dy + PH, dx:]

    s1 = np.zeros((72, 128), f32)
    for dy in range(3):
        for dx in range(3):
            for h in range(8):
                s1[(dy * 3 + dx) * 8 + h, 16 * h:16 * h + C1] = \
                    conv1_w[:, 0, dy, dx]

    # conv2 stationaries [6, 128, 128]: pass t = dy*2 + grp;
    # partition p = e*64 + img*16 + ch supplies tap dx = 2*grp + e
    s2 = np.zeros((6, 128, 128), f32)
    for t in range(6):
        dy, grp = t // 2, t % 2
        for e in range(2):
            dx = 2 * grp + e
            if dx > 2:
                continue
            for img in range(4):
                for ch in range(C1):
                    s2[t, 64 * e + 16 * img + ch, 32 * img:32 * img + C2] = \
                        conv2_w[:, ch, dy, dx]

    # fc1 weight [128, 99, 512]: tile k row p = feature 128k+p of the
    # core's shard; tile 98 row 0 = fc1_b/8 (ones-column bias trick).
    b1_8 = np.asarray(fc1_b, f32) / N_CORES
    w1ts = []
    for c in range(N_CORES):
        idx = w_idx1[:, FSH * c:FSH * (c + 1)]             # [512, 12544]
        wt = np.zeros((128, NKB, H1), f32)
        wt[:, :NK, :] = values[idx].T.reshape(NK, 128, H1).transpose(1, 0, 2)
        wt[0, NK, :] = b1_8
        w1ts.append(np.ascontiguousarray(
            wt.reshape(128, NKB * H1)).astype(BF16NP))

    w2 = np.ascontiguousarray(values[w_idx2]).astype(f32)     # [4, 512]
    w2r = np.broadcast_to(w2[None, :, :], (4, NOUT, H1)).copy()
    b2t = np.broadcast_to(np.asarray(fc2_b, f32), (4, 4)).copy()

    cb1 = np.zeros((128, 1), f32)
    for h in range(8):
        cb1[16 * h:16 * h + C1, 0] = np.asarray(conv1_b, f32)
    cb2 = np.zeros((128, 1), f32)
    for img in range(4):
        cb2[32 * img:32 * img + C2, 0] = np.asarray(conv2_b, f32)

    s1 = s1.astype(BF16NP)
    s2 = s2.astype(BF16NP)
    in_maps = []
    for c in range(N_CORES):
        in_maps.append({
            "x9": np.ascontiguousarray(x9[c]).astype(BF16NP),
            "s1": s1, "s2": s2,
            "w1t": w1ts[c],
            "w2r": w2r, "b2t": b2t,
            "cb1": cb1, "cb2": cb2,
        })
    return in_maps


def kernel(x, conv1_w, conv1_b, conv2_w, conv2_b, values, w_idx1, fc1_b,
           w_idx2, fc2_b, _trace=False, _trace_kwargs=None):
    nc = _get_program()
    in_maps = _host_prep(x, conv1_w, conv1_b, conv2_w, conv2_b, values,
                         w_idx1, fc1_b, w_idx2, fc2_b)
    res = run_bass_kernel_spmd(nc, in_maps, core_ids=list(range(N_CORES)),
                               trace=_trace, **(_trace_kwargs or {}))
    out = np.zeros((B, NOUT), np.float32)
    for c in range(N_CORES):
        out[4 * c:4 * c + 4] = res.results[c]["out"]
    if _trace:
        kernel.last_result = res
    return out


if __name__ == "__main__":
    rng = np.random.default_rng(0)
    ins = {
        "x": rng.standard_normal((B, 1, IMG, IMG), dtype=np.float32),
        "conv1_w": rng.standard_normal((16, 1, 3, 3), dtype=np.float32) * 0.1,
        "conv1_b": np.zeros(16, np.float32),
        "conv2_w": rng.standard_normal((32, 16, 3, 3), dtype=np.float32) * 0.05,
        "conv2_b": np.zeros(32, np.float32),
        "values": np.sort(rng.standard_normal(4096).astype(np.float32) * 0.01),
        "w_idx1": rng.integers(0, 4096, (512, FEAT), dtype=np.int32),
        "fc1_b": np.zeros(512, np.float32),
        "w_idx2": rng.integers(0, 4096, (4, 512), dtype=np.int32),
        "fc2_b": np.zeros(4, np.float32),
    }
    out = kernel(**ins)
    print("out shape", out.shape, "sample row", out[0])


# revision 48
# speedup vs baseline: 1.7747x; 1.4073x over previous
"""Trainium2 Bass kernel for nn_MemristorCNN (embedding_lookup, 8 cores).

Strategy (per sharding hint):
- Host gathers the codebook weight W1 = values[w_idx1] and ships the
  *gathered weight* in bf16, column-sharded over in_features (12544
  features = 4 conv2 output channels per core), laid out [128, 99, 512]
  so the whole 12.9 MB stream is ONE DMA instruction (issue-rate, not
  bandwidth, limited the old 98-DMA stream); tile k=98 carries
  fc1_bias/8 against an on-device ones-column so the bias rides the
  PSUM accumulation and survives the ReduceScatter sum.
- Conv stack runs data-parallel (4 images per core); conv1 packs
  (tap, half-image) into K=72 with dx pre-shifted on host; conv2 packs
  (image, channel, dx-pair) into K=128 with 6 tap passes.
- AllToAll redistributes conv output h from batch-sharded to
  feature-sharded with an img-major payload so the receive buffer is a
  contiguous [32, 12544] view; ONE xbar transpose-DMA (out[p,k,i] =
  in[i, 128k+p]) lands it feature-major in SBUF, replacing 98 PE
  transposes; fc1 accumulates 99 matmuls; ReduceScatter sums partials;
  relu + fc2 (DVE mult+reduce) finish on device and the host
  concatenates the per-core [4, 4] outputs.
"""

import sys

import numpy as np
import ml_dtypes

BF16NP = ml_dtypes.bfloat16

for _p in ("/opt/trn_rl_repo",):
    if _p not in sys.path:
        sys.path.insert(0, _p)

import concourse.bacc as bacc
import concourse.bass as bass  # noqa: F401
import concourse.tile as tile
from concourse import mybir
from concourse.bass_utils import run_bass_kernel_spmd

F32 = mybir.dt.float32
BF16 = mybir.dt.bfloat16
FP8 = mybir.dt.float8e4
W1SCALE = 4096.0
RELU = mybir.ActivationFunctionType.Relu
COPY = mybir.ActivationFunctionType.Copy

N_CORES = 8
B = 32
IMG = 224
C1, C2 = 16, 32
PH, PW = 112, 112
HH, HW = 56, 56
FEAT = C2 * HH * HW          # 100352
FSH = FEAT // N_CORES        # 12544
NK = FSH // 128              # 98
NKB = NK + 1                 # +1 bias tile
H1 = 512
NOUT = 4

_CACHE = {}


def _build_program(stop_after: str = 'full'):
    nc = bacc.Bacc("TRN2", target_bir_lowering=False, debug=False,
                   num_devices=N_CORES)
    _emit(nc, stop_after)
    nc.compile()
    return nc


def _emit(nc, stop_after: str):
    # ---- kernel I/O ----
    x9_t = nc.dram_tensor("x9", [72, PH, 232], BF16, kind="ExternalInput")
    s1_t = nc.dram_tensor("s1", [72, 128], BF16, kind="ExternalInput")
    s2_t = nc.dram_tensor("s2", [6, 128, 128], BF16, kind="ExternalInput")
    w1t_t = nc.dram_tensor("w1t", [128, NKB * H1], FP8, kind="ExternalInput")
    w2c_t = nc.dram_tensor("w2c", [128, 4, NOUT], F32, kind="ExternalInput")
    ident4_t = nc.dram_tensor("ident4", [4, 4], F32, kind="ExternalInput")
    b2t_t = nc.dram_tensor("b2t", [4, 4], F32, kind="ExternalInput")
    cb1_t = nc.dram_tensor("cb1", [128, 1], F32, kind="ExternalInput")
    cb2_t = nc.dram_tensor("cb2", [128, 1], F32, kind="ExternalInput")
    if stop_after in ("dumphT", "dumpw", "dumpfc1"):
        shp = {"dumphT": [128, NKB, 32], "dumpw": [128, NKB, H1],
               "dumpfc1": [B, H1]}[stop_after]
        out_t = nc.dram_tensor("out", shp, F32, kind="ExternalOutput")
    elif stop_after in ("dumpa2ain", "dumpa2aout"):
        out_t = nc.dram_tensor("out", [B, FSH], BF16, kind="ExternalOutput")
    elif stop_after == "dumph":
        out_t = nc.dram_tensor("out", [128, 3136], BF16, kind="ExternalOutput")
    else:
        out_t = nc.dram_tensor("out", [4, NOUT], F32, kind="ExternalOutput")

    # ---- internal DRAM (collective bounce buffers) ----
    a2a_in = nc.dram_tensor("a2a_in", [B, FSH], BF16)
    a2a_out = nc.dram_tensor("a2a_out", [N_CORES, 4, FSH], BF16)
    rs_in = nc.dram_tensor("rs_in", [B, H1], F32)
    rs_out = nc.dram_tensor("rs_out", [4, H1], F32)
    sync_in = nc.dram_tensor("sync_in", [8, 4], F32)
    sync_out = nc.dram_tensor("sync_out", [1, 4], F32)

    groups = [list(range(N_CORES))]

    with tile.TileContext(nc) as tc:
        with (
            tc.tile_pool(name="const", bufs=1) as cpool,
            tc.tile_pool(name="ps", bufs=4, space="PSUM") as pspool,
            tc.tile_pool(name="work", bufs=2) as wkpool,
            tc.tile_pool(name="xin", bufs=3) as xpool,
            tc.tile_pool(name="persist", bufs=1) as pers,
        ):
            # -------- latency-critical loads first --------
            # conv1 input: partition (dy*3+dx)*8 + h holds
            # x_pad[img(h), y0(h)+dy+y, dx+c]; 14 8-row chunks,
            # triple-buffered.
            NXC = 14
            x9_tiles = []
            for e in range(NXC):
                x9e = xpool.tile([72, 8, 232], BF16, tag="x9")
                if e == 0:
                    nc.sync.dma_start(out=x9e[:, :, :],
                                      in_=x9_t[:, 0:8, :])
                x9_tiles.append(x9e)
            s1_sb = cpool.tile([72, 128], BF16, tag="s1")
            nc.sync.dma_start(out=s1_sb[:, :], in_=s1_t[:, :])
            for e in range(1, NXC):
                nc.sync.dma_start(out=x9_tiles[e][:, :, :],
                                  in_=x9_t[:, 8 * e:8 * e + 8, :])

            # small constants first on the scalar ring (FIFO per ring —
            # anything queued after the big w1t DMA drains after it)
            cb1_sb = cpool.tile([128, 1], F32, tag="cb1")
            nc.scalar.dma_start(out=cb1_sb[:, :], in_=cb1_t[:, :])
            cb2_sb = cpool.tile([128, 1], F32, tag="cb2")
            nc.scalar.dma_start(out=cb2_sb[:, :], in_=cb2_t[:, :])
            s2_sb = cpool.tile([128, 6, 128], BF16, tag="s2")
            nc.scalar.dma_start(out=s2_sb[:, :, :],
                                in_=s2_t[:, :, :].rearrange("t p m -> p t m"))

            w2c_sb = cpool.tile([128, 4, NOUT], F32, tag="w2c")
            nc.scalar.dma_start(out=w2c_sb[:, :, :], in_=w2c_t[:, :, :])
            ident4_sb = cpool.tile([4, 4], F32, tag="id4")
            nc.scalar.dma_start(out=ident4_sb[:, :], in_=ident4_t[:, :])
            b2t_sb = cpool.tile([4, 4], F32, tag="b2t")
            nc.scalar.dma_start(out=b2t_sb[:, :], in_=b2t_t[:, :])

            # fc1 weight stream: 13 chunked DMAs, each gated on conv
            # progress via a 1-element ACT write into its destination
            # (WAW dep) — weights (needed only by fc1, ~100 us in) drain
            # paced by conv instead of starving the conv-critical
            # x9/repack traffic.  Chunk DMAs are emitted inside the conv
            # loops; see _wchunk().
            WCH = 8                         # k-tiles per chunk
            wch_edges = list(range(0, NKB, WCH)) + [NKB]
            have_w = stop_after not in ("dumpa2ain", "dumpa2aout")
            if have_w:
                wts = pers.tile([128, NKB, H1], FP8, tag="w1")
                wflat = wts[:, :, :].rearrange("p k m -> p (k m)")

            def _wchunk(i, gate_src):
                if not have_w or i >= len(wch_edges) - 1:
                    return
                k0, k1 = wch_edges[i], wch_edges[i + 1]
                nc.scalar.activation(wts[0:1, k0, 0:1], gate_src, COPY)
                nc.sync.dma_start(out=wflat[:, H1 * k0:H1 * k1],
                                  in_=w1t_t[:, H1 * k0:H1 * k1])

            # align the 8 cores up front (PJRT launch skew would otherwise
            # surface as a long wait inside the first data collective);
            # the conv phase runs during the alignment, absorbing it.
            nc.gpsimd.collective_compute(
                "ReduceScatter", mybir.AluOpType.add,
                replica_groups=groups,
                ins=[sync_in[:, :]], outs=[sync_out[:, :]])

            # conv2 input buffer: partition e*64 + img*16 + ch holds the
            # padded channel image, dx-shifted by e.  Repack fills rows
            # 1..112 full-width; only rows 0/113 need zeroing.
            c2in = pers.tile([128, 114, 117], BF16, tag="bigC")
            nc.gpsimd.memset(c2in[:, 0, :], 0.0)
            nc.gpsimd.memset(c2in[:, 113, :], 0.0)

            # pool1 rows are 117 wide with zeroed borders (cols 0,
            # 113-116 + one spare element) so the repack shifts by e via
            # a single contiguous flat copy per (chunk, half, e).
            pool1_a = pers.tile([128, 28 * 117 + 1], BF16, tag="bigB1")
            pool1_b = pers.tile([128, 28 * 117 + 1], BF16, tag="bigB2")
            pool1_parts = [pool1_a, pool1_b]
            pool1_views = []
            for t in pool1_parts:
                pv = t[:, 0:28 * 117].rearrange("p (r c) -> p r c", c=117)
                nc.gpsimd.memset(pv[:, :, 0], 0.0)
                nc.gpsimd.memset(pv[:, :, 113:117], 0.0)
                nc.gpsimd.memset(t[:, 28 * 117:], 0.0)
                pool1_views.append(pv)

            # fc1 bias rides k-tile 98: ones column on partition 0
            hT = pers.tile([128, NKB, 32], BF16, tag="bigHT")
            nc.gpsimd.memset(hT[:, NK, :], 0.0)
            nc.gpsimd.memset(hT[0:1, NK, :], 1.0)

            # ---------------- conv1 + pool1 + relu ----------------
            # out partition m = h*16 + oc = half*64 + img*16 + oc
            for T in range(28):            # 2 pooled rows per psum tile
                ps = pspool.tile([128, 2, 512], F32, tag="ps")
                # de-interleave the 2x2 pool pairs in the PSUM AP: matmul
                # column j = (r, x, w) lands at offset 224r + 112w + x,
                # so the pool chain reads contiguous 112-runs.
                v = ps[:, :, 0:448].rearrange("p g (r w x) -> p g r x w",
                                              r=2, w=2)
                for g in range(2):
                    yp = T * 2 + g         # pooled row within half
                    e, ypl = yp // 4, yp % 4
                    rhs = x9_tiles[e][:, 2 * ypl:2 * ypl + 2,
                                      :224].rearrange(
                        "p r (x w) -> p r w x", w=2)
                    nc.tensor.matmul(
                        ps[:, g, 0:448],
                        lhsT=s1_sb[:, :],
                        rhs=rhs,
                        start=True, stop=True)
                c1 = wkpool.tile([128, 2, 2, 112], F32, tag="mc")
                nc.scalar.activation(c1[:, :, :, :], v[:, :, :, :, 1], COPY)
                m1 = wkpool.tile([128, 2, 2, 112], F32, tag="mx")
                nc.vector.tensor_max(m1[:, :, :, :], v[:, :, :, :, 0],
                                     c1[:, :, :, :])
                m2 = wkpool.tile([128, 2, 112], F32, tag="mxb")
                nc.vector.tensor_max(m2[:, :, :], m1[:, :, 0, :],
                                     m1[:, :, 1, :])
                half_t, row_t = divmod(2 * T, 28)
                nc.scalar.activation(
                    pool1_views[half_t][:, row_t:row_t + 2, 1:113],
                    m2[:, :, :], RELU, bias=cb1_sb[:, :])

            if stop_after == "conv1":
                dbg = wkpool.tile([4, NOUT], F32, tag="outsb")
                nc.vector.tensor_copy(dbg[:, :], pool1_views[0][0:4, 0, 1:5])
                nc.sync.dma_start(out=out_t[:, :], in_=dbg[:, :])
                return

            # -------- repack pool1 -> conv2 input (padded, merged halves,
            # two dx-shifted copies); 8 flat contiguous DMAs (one
            # descriptor per partition).  c2in[R0+r, c] = pool[r, c+e];
            # the pool's zeroed borders supply the conv padding.
            c2flat = c2in[:, :, :].rearrange("p r c -> p (r c)")
            for chunk in range(2):
                for half in range(2):
                    r0 = 117 * (56 * half + 1 + 28 * chunk)
                    for e in range(2):
                        nc.scalar.dma_start(
                            out=c2flat[64 * e:64 * e + 64,
                                       r0:r0 + 28 * 117],
                            in_=pool1_parts[chunk][64 * half:64 * half + 64,
                                                   e:e + 28 * 117])

            # ---------------- conv2 + pool2 + relu ----------------
            # out partition m = img*32 + oc; 6 passes t=(dy, grp):
            # partition block e supplies tap dx = 2*grp + e.
            h_sb = pers.tile([128, 7, 4, 2, 56], BF16, tag="bigD")
            for T in range(14):            # 8 conv rows / 4 pooled rows
                ps = pspool.tile([128, 2, 512], F32, tag="ps")
                # de-interleaved pool pairs: column j = (r, x, w) lands at
                # offset 112r + 56w + x (contiguous 56-runs for the pool).
                v = ps[:, :, 0:448].rearrange("p s (r w x) -> p s r x w",
                                              r=4, w=2)
                for sub in range(2):
                    y0 = 8 * T + 4 * sub
                    for t in range(6):
                        dy, grp = t // 2, t % 2
                        rhs = c2in[:, y0 + dy:y0 + dy + 4,
                                   2 * grp:2 * grp + 112].rearrange(
                            "p r (x w) -> p r w x", w=2)
                        nc.tensor.matmul(
                            ps[:, sub, 0:448],
                            lhsT=s2_sb[:, t, :],
                            rhs=rhs,
                            start=(t == 0), stop=(t == 5))
                c1 = wkpool.tile([128, 2, 4, 56], F32, tag="mc2")
                nc.scalar.activation(c1[:, :, :, :], v[:, :, :, :, 1], COPY)
                m1 = wkpool.tile([128, 2, 4, 56], F32, tag="mx2")
                nc.vector.tensor_max(m1[:, :, :, :], v[:, :, :, :, 0],
                                     c1[:, :, :, :])
                v2 = m1[:, :, :, :].rearrange("p s (rp w) x -> p s rp w x",
                                              w=2)
                m2 = wkpool.tile([128, 2, 2, 56], F32, tag="mxb2")
                nc.vector.tensor_max(m2[:, :, :, :], v2[:, :, :, 0, :],
                                     v2[:, :, :, 1, :])
                # pooled rows 4T..4T+4 -> h_sb[T//2, 2*(T%2) + (0..1), ...]
                nc.scalar.activation(
                    h_sb[:, T // 2, 2 * (T % 2):2 * (T % 2) + 2, :, :],
                    m2[:, :, :, :], RELU, bias=cb2_sb[:, :])
                # weight chunks gated on conv2 progress (conv1/repack get
                # the DMA bandwidth until conv2 is consuming)
                _wchunk(T, h_sb[0:1, T // 2, 2 * (T % 2), 0, 0:1])

            if stop_after == "conv2":
                dbg = wkpool.tile([4, NOUT], F32, tag="outsb")
                nc.vector.tensor_copy(dbg[:, :], h_sb[0:4, 0, 0, 0, 0:4])
                nc.sync.dma_start(out=out_t[:, :], in_=dbg[:, :])
                return

            # -------- AllToAll: batch-shard -> feature-shard --------
            # payload row (4*dest + img_loc), col (ch_loc, sp): receiver
            # gets a contiguous [32, 12544] img-major view.  One DMA per
            # local image (DMA APs are limited to 3 dims).
            h_flat = h_sb[:, :, :, :, :].rearrange(
                "(i m) t j r x -> i m (t j r x)", i=4)
            for il in range(4):
                nc.scalar.dma_start(
                    out=a2a_in[:, :].rearrange(
                        "(d i) (c s) -> i d c s", d=8, c=4)[il],
                    in_=h_flat[il])
            if stop_after == "dumph":
                nc.sync.dma_start(
                    out=out_t[:, :],
                    in_=h_sb[:, :, :, :, :].rearrange("p t j r x -> p (t j r x)"))
                return

            if stop_after == "dumpa2ain":
                dbg = pers.tile([32, FSH], BF16, tag="dmpa")
                nc.sync.dma_start(out=dbg[:, :], in_=a2a_in[:, :])
                nc.sync.dma_start(out=out_t[:, :], in_=dbg[:, :])
                return

            nc.gpsimd.collective_compute(
                "AllToAll", mybir.AluOpType.bypass, replica_groups=groups,
                ins=[a2a_in[:, :]], outs=[a2a_out[:, :, :]])

            if stop_after == "dumpa2aout":
                dbg = pers.tile([32, FSH], BF16, tag="dmpa")
                nc.sync.dma_start(
                    out=dbg[:, :],
                    in_=a2a_out[:, :, :].rearrange("s i f -> (s i) f"))
                nc.sync.dma_start(out=out_t[:, :], in_=dbg[:, :])
                return

            # -------- ONE xbar transpose-DMA -> feature-major hT --------
            # out[p, k, i] = in[i, 128k + p]
            nc.scalar.dma_start(
                out=hT[:, 0:NK, :],
                in_=a2a_out[:, :, :].rearrange("s i f -> (s i) f"),
                transpose=True)

            if stop_after == "a2a":
                dbg = wkpool.tile([4, NOUT], F32, tag="outsb")
                nc.vector.tensor_copy(dbg[:, :], hT[0:4, 0, 0:4])
                nc.sync.dma_start(out=out_t[:, :], in_=dbg[:, :])
                return

            if stop_after in ("dumphT", "dumpw"):
                src = hT if stop_after == "dumphT" else wts
                n = 32 if stop_after == "dumphT" else H1
                for k in range(NKB):
                    dbg = wkpool.tile([128, n], F32, tag="dmp")
                    nc.vector.tensor_copy(dbg[:, :], src[:, k, :])
                    nc.sync.dma_start(out=out_t[:, k, :], in_=dbg[:, :])
                return

            # ---------------- fc1 partial ----------------
            # 4 concurrent matmuls in 32-column PE groups (col tiling):
            # col group j accumulates k-tiles j, j+4, j+8, ... into PSUM
            # rows 32j..32j+32; the 4 row blocks are summed afterwards.
            fc1_ps = pspool.tile([128, H1], F32, tag="ps")
            NG = (NKB + 3) // 4
            for g in range(NG):
                for j in range(4):
                    k = 4 * g + j
                    if k >= NKB:
                        continue
                    nc.tensor.matmul(fc1_ps[32 * j:32 * j + 32, :],
                                     lhsT=hT[:, k, :], rhs=wts[:, k, :],
                                     tile_position=(0, 32 * j),
                                     start=(g == 0),
                                     stop=(k + 4 >= NKB))
            pa = wkpool.tile([B, H1], F32, tag="fc1a")
            nc.scalar.activation(pa[:, :], fc1_ps[32:64, :], COPY)
            pb = wkpool.tile([B, H1], F32, tag="fc1b")
            nc.scalar.activation(pb[:, :], fc1_ps[96:128, :], COPY)
            sa = wkpool.tile([B, H1], F32, tag="fc1c")
            nc.vector.tensor_add(sa[:, :], fc1_ps[0:32, :], pa[:, :])
            sb = wkpool.tile([B, H1], F32, tag="fc1d")
            nc.vector.tensor_add(sb[:, :], fc1_ps[64:96, :], pb[:, :])
            fc1_sb = wkpool.tile([B, H1], F32, tag="fc1")
            nc.vector.tensor_add(fc1_sb[:, :], sa[:, :], sb[:, :])
            nc.scalar.activation(fc1_sb[:, :], fc1_sb[:, :], COPY,
                                 scale=1.0 / W1SCALE)
            nc.scalar.dma_start(out=rs_in[:, :], in_=fc1_sb[:, :])

            if stop_after == "fc1":
                nc.sync.dma_start(out=out_t[:, :], in_=fc1_sb[0:4, 0:4])
                return

            # -------- ReduceScatter + relu + fc2 --------
            nc.gpsimd.collective_compute(
                "ReduceScatter", mybir.AluOpType.add, replica_groups=groups,
                ins=[rs_in[:, :]], outs=[rs_out[:, :]])

            h1 = wkpool.tile([4, H1], F32, tag="h1")
            nc.scalar.dma_start(out=h1[:, :], in_=rs_out[:, :])
            nc.scalar.activation(h1[:, :], h1[:, :], RELU)

            if stop_after == "rs":
                dbg = wkpool.tile([4, NOUT], F32, tag="outsb")
                nc.vector.tensor_copy(dbg[:, :], h1[0:4, 0:4])
                nc.sync.dma_start(out=out_t[:, :], in_=dbg[:, :])
                return

            # fc2 on the PE: transpose h1 into [128, 4kb, 4i] via 4 PE
            # transposes, then 4 accumulating [128,4i]x[128,4o] matmuls.
            h1t_ps = pspool.tile([128, 4, 4], F32, tag="ps")
            for kb in range(4):
                nc.tensor.transpose(h1t_ps[:, kb, :],
                                    h1[:, 128 * kb:128 * kb + 128],
                                    ident4_sb[:, :])
            h1t = wkpool.tile([128, 4, 4], F32, tag="h1t")
            nc.vector.tensor_copy(h1t[:, :, :], h1t_ps[:, :, :])
            fc2_ps = pspool.tile([4, NOUT], F32, tag="ps")
            for kb in range(4):
                nc.tensor.matmul(fc2_ps[:, :], lhsT=h1t[:, kb, :],
                                 rhs=w2c_sb[:, kb, :],
                                 start=(kb == 0), stop=(kb == 3))
            out_sb = wkpool.tile([4, NOUT], F32, tag="outsb")
            nc.vector.tensor_add(out_sb[:, :], fc2_ps[:, :], b2t_sb[:, :])
            nc.scalar.dma_start(out=out_t[:, :], in_=out_sb[:, :])


def _get_program():
    key = ("prog",)
    if key not in _CACHE:
        _CACHE[key] = _build_program()
    return _CACHE[key]


def _host_prep(x, conv1_w, conv1_b, conv2_w, conv2_b, values, w_idx1,
               fc1_b, w_idx2, fc2_b):
    """Build per-core input maps (numpy, bf16 for PE-facing tensors)."""
    f32 = np.float32
    x = np.asarray(x, f32)
    conv1_w = np.asarray(conv1_w, f32)
    conv2_w = np.asarray(conv2_w, f32)
    values = np.asarray(values, f32)
    w_idx1 = np.asarray(w_idx1)
    w_idx2 = np.asarray(w_idx2)

    x_pad = np.zeros((B, 226, 232), f32)
    x_pad[:, 1:225, 1:225] = x[:, 0]

    # x9[c]: [72, 112, 232]; partition (dy*3+dx)*8 + h, h = 4*half + img_loc
    x9 = np.zeros((N_CORES, 72, PH, 232), f32)
    for dy in range(3):
        for dx in range(3):
            for h in range(8):
                half, il = h // 4, h % 4
                y0 = PH * half
                for c in range(N_CORES):
                    x9[c, (dy * 3 + dx) * 8 + h, :, :232 - dx] = \
                        x_pad[4 * c + il, y0 + dy:y0 + dy + PH, dx:]

    s1 = np.zeros((72, 128), f32)
    for dy in range(3):
        for dx in range(3):
            for h in range(8):
                s1[(dy * 3 + dx) * 8 + h, 16 * h:16 * h + C1] = \
                    conv1_w[:, 0, dy, dx]

    # conv2 stationaries [6, 128, 128]: pass t = dy*2 + grp;
    # partition p = e*64 + img*16 + ch supplies tap dx = 2*grp + e
    s2 = np.zeros((6, 128, 128), f32)
    for t in range(6):
        dy, grp = t // 2, t % 2
        for e in range(2):
            dx = 2 * grp + e
            if dx > 2:
                continue
            for img in range(4):
                for ch in range(C1):
                    s2[t, 64 * e + 16 * img + ch, 32 * img:32 * img + C2] = \
                        conv2_w[:, ch, dy, dx]

    # fc1 weight [128, 99, 512] in fp8e4m3 scaled by W1SCALE: tile k
    # row p = feature 128k+p of the core's shard; tile 98 row 0 =
    # fc1_b/8 (ones-column bias trick; rides the same scale).
    b1_8 = np.asarray(fc1_b, f32) / N_CORES
    w1ts = []
    for c in range(N_CORES):
        idx = w_idx1[:, FSH * c:FSH * (c + 1)]             # [512, 12544]
        wt = np.zeros((128, NKB, H1), f32)
        wt[:, :NK, :] = values[idx].T.reshape(NK, 128, H1).transpose(1, 0, 2)
        wt[0, NK, :] = b1_8
        wq = np.clip(wt * W1SCALE, -224.0, 224.0)
        w1ts.append(np.ascontiguousarray(
            wq.reshape(128, NKB * H1)).astype(ml_dtypes.float8_e4m3fn))

    w2 = np.ascontiguousarray(values[w_idx2]).astype(f32)     # [4, 512]
    # w2c[p, kb, o] = W2[o, 128*kb + p]
    w2c = np.ascontiguousarray(w2.T.reshape(4, 128, 4).transpose(1, 0, 2))
    ident4 = np.eye(4, dtype=f32)
    b2t = np.broadcast_to(np.asarray(fc2_b, f32), (4, 4)).copy()

    cb1 = np.zeros((128, 1), f32)
    for h in range(8):
        cb1[16 * h:16 * h + C1, 0] = np.asarray(conv1_b, f32)
    cb2 = np.zeros((128, 1), f32)
    for img in range(4):
        cb2[32 * img:32 * img + C2, 0] = np.asarray(conv2_b, f32)

    s1 = s1.astype(BF16NP)
    s2 = s2.astype(BF16NP)
    in_maps = []
    for c in range(N_CORES):
        in_maps.append({
            "x9": np.ascontiguousarray(x9[c]).astype(BF16NP),
            "s1": s1, "s2": s2,
            "w1t": w1ts[c],
            "w2c": w2c, "ident4": ident4, "b2t": b2t,
            "cb1": cb1, "cb2": cb2,
        })
    return in_maps


def kernel(x, conv1_w, conv1_b, conv2_w, conv2_b, values, w_idx1, fc1_b,
           w_idx2, fc2_b, _trace=False, _trace_kwargs=None):
    nc = _get_program()
    in_maps = _host_prep(x, conv1_w, conv1_b, conv2_w, conv2_b, values,
                         w_idx1, fc1_b, w_idx2, fc2_b)
    res = run_bass_kernel_spmd(nc, in_maps, core_ids=list(range(N_CORES)),
                               trace=_trace, **(_trace_kwargs or {}))
    out = np.zeros((B, NOUT), np.float32)
    for c in range(N_CORES):
        out[4 * c:4 * c + 4] = res.results[c]["out"]
    if _trace:
        kernel.last_result = res
    return out


if __name__ == "__main__":
    rng = np.random.default_rng(0)
    ins = {
        "x": rng.standard_normal((B, 1, IMG, IMG), dtype=np.float32),
        "conv1_w": rng.standard_normal((16, 1, 3, 3), dtype=np.float32) * 0.1,
        "conv1_b": np.zeros(16, np.float32),
        "conv2_w": rng.standard_normal((32, 16, 3, 3), dtype=np.float32) * 0.05,
        "conv2_b": np.zeros(32, np.float32),
        "values": np.sort(rng.standard_normal(4096).astype(np.float32) * 0.01),
        "w_idx1": rng.integers(0, 4096, (512, FEAT), dtype=np.int32),
        "fc1_b": np.zeros(512, np.float32),
        "w_idx2": rng.integers(0, 4096, (4, 512), dtype=np.int32),
        "fc2_b": np.zeros(4, np.float32),
    }
    out = kernel(**ins)
    print("out shape", out.shape, "sample row", out[0])


# revision 52
# speedup vs baseline: 1.9032x; 1.0724x over previous
"""Trainium2 Bass kernel for nn_MemristorCNN (embedding_lookup, 8 cores).

Strategy (per sharding hint):
- Host gathers the codebook weight W1 = values[w_idx1] and ships the
  *gathered weight* in bf16, column-sharded over in_features (12544
  features = 4 conv2 output channels per core), laid out [128, 99, 512]
  so the whole 12.9 MB stream is ONE DMA instruction (issue-rate, not
  bandwidth, limited the old 98-DMA stream); tile k=98 carries
  fc1_bias/8 against an on-device ones-column so the bias rides the
  PSUM accumulation and survives the ReduceScatter sum.
- Conv stack runs data-parallel (4 images per core); conv1 packs
  (tap, half-image) into K=72 with dx pre-shifted on host; conv2 packs
  (image, channel, dx-pair) into K=128 with 6 tap passes.
- AllToAll redistributes conv output h from batch-sharded to
  feature-sharded with an img-major payload so the receive buffer is a
  contiguous [32, 12544] view; ONE xbar transpose-DMA (out[p,k,i] =
  in[i, 128k+p]) lands it feature-major in SBUF, replacing 98 PE
  transposes; fc1 accumulates 99 matmuls; ReduceScatter sums partials;
  relu + fc2 (DVE mult+reduce) finish on device and the host
  concatenates the per-core [4, 4] outputs.
"""

import sys

import numpy as np
import ml_dtypes

BF16NP = ml_dtypes.bfloat16

for _p in ("/opt/trn_rl_repo",):
    if _p not in sys.path:
        sys.path.insert(0, _p)

import concourse.bacc as bacc
import concourse.bass as bass  # noqa: F401
import concourse.tile as tile
from concourse import mybir
from concourse.bass_utils import run_bass_kernel_spmd

F32 = mybir.dt.float32
BF16 = mybir.dt.bfloat16
FP8 = mybir.dt.float8e4
W1SCALE = 4096.0
RELU = mybir.ActivationFunctionType.Relu
COPY = mybir.ActivationFunctionType.Copy

N_CORES = 8
B = 32
IMG = 224
C1, C2 = 16, 32
PH, PW = 112, 112
HH, HW = 56, 56
FEAT = C2 * HH * HW          # 100352
FSH = FEAT // N_CORES        # 12544
NK = FSH // 128              # 98
NKB = NK + 1                 # +1 bias tile
H1 = 512
NOUT = 4

_CACHE = {}


def _build_program(stop_after: str = 'full'):
    nc = bacc.Bacc("TRN2", target_bir_lowering=False, debug=False,
                   num_devices=N_CORES)
    _emit(nc, stop_after)
    nc.compile()
    return nc


def _emit(nc, stop_after: str):
    # ---- kernel I/O ----
    x9_t = nc.dram_tensor("x9", [72, PH, 232], BF16, kind="ExternalInput")
    s1_t = nc.dram_tensor("s1", [72, 128], BF16, kind="ExternalInput")
    s2_t = nc.dram_tensor("s2", [6, 128, 128], BF16, kind="ExternalInput")
    w1t_t = nc.dram_tensor("w1t", [128, NKB * H1], FP8, kind="ExternalInput")
    w2c_t = nc.dram_tensor("w2c", [128, 4, NOUT], F32, kind="ExternalInput")
    ident4_t = nc.dram_tensor("ident4", [4, 4], F32, kind="ExternalInput")
    b2t_t = nc.dram_tensor("b2t", [4, 4], F32, kind="ExternalInput")
    cb1_t = nc.dram_tensor("cb1", [128, 1], F32, kind="ExternalInput")
    cb2_t = nc.dram_tensor("cb2", [128, 1], F32, kind="ExternalInput")
    if stop_after in ("dumphT", "dumpw", "dumpfc1"):
        shp = {"dumphT": [128, NKB, 32], "dumpw": [128, NKB, H1],
               "dumpfc1": [B, H1]}[stop_after]
        out_t = nc.dram_tensor("out", shp, F32, kind="ExternalOutput")
    elif stop_after == "dumph":
        out_t = nc.dram_tensor("out", [128, 3136], BF16, kind="ExternalOutput")
    else:
        out_t = nc.dram_tensor("out", [4, NOUT], F32, kind="ExternalOutput")

    # ---- internal DRAM (collective bounce buffers) ----
    FS1, FS2 = 4 * 32 * 56, 4 * 24 * 56        # 7168 + 5376 = FSH
    NK1 = FS1 // 128                             # 56 k-tiles in phase 1
    a2a_in1 = nc.dram_tensor("a2a_in1", [B, FS1], BF16)
    a2a_out1 = nc.dram_tensor("a2a_out1", [N_CORES, 4, FS1], BF16)
    a2a_in2 = nc.dram_tensor("a2a_in2", [B, FS2], BF16)
    a2a_out2 = nc.dram_tensor("a2a_out2", [N_CORES, 4, FS2], BF16)
    rs_in = nc.dram_tensor("rs_in", [B, H1], F32)
    rs_out = nc.dram_tensor("rs_out", [4, H1], F32)
    sync_in = nc.dram_tensor("sync_in", [8, 4], F32)
    sync_out = nc.dram_tensor("sync_out", [1, 4], F32)

    groups = [list(range(N_CORES))]

    with tile.TileContext(nc) as tc:
        with (
            tc.tile_pool(name="const", bufs=1) as cpool,
            tc.tile_pool(name="ps", bufs=4, space="PSUM") as pspool,
            tc.tile_pool(name="work", bufs=2) as wkpool,
            tc.tile_pool(name="xin", bufs=3) as xpool,
            tc.tile_pool(name="persist", bufs=1) as pers,
        ):
            # -------- latency-critical loads first --------
            # conv1 input: partition (dy*3+dx)*8 + h holds
            # x_pad[img(h), y0(h)+dy+y, dx+c]; 14 8-row chunks,
            # triple-buffered.
            NXC = 14
            x9_tiles = []
            for e in range(NXC):
                x9e = xpool.tile([72, 8, 232], BF16, tag="x9")
                if e == 0:
                    nc.sync.dma_start(out=x9e[:, :, :],
                                      in_=x9_t[:, 0:8, :])
                x9_tiles.append(x9e)
            s1_sb = cpool.tile([72, 128], BF16, tag="s1")
            nc.sync.dma_start(out=s1_sb[:, :], in_=s1_t[:, :])
            for e in range(1, NXC):
                nc.sync.dma_start(out=x9_tiles[e][:, :, :],
                                  in_=x9_t[:, 8 * e:8 * e + 8, :])

            # small constants first on the scalar ring (FIFO per ring —
            # anything queued after the big w1t DMA drains after it)
            cb1_sb = cpool.tile([128, 1], F32, tag="cb1")
            nc.scalar.dma_start(out=cb1_sb[:, :], in_=cb1_t[:, :])
            cb2_sb = cpool.tile([128, 1], F32, tag="cb2")
            nc.scalar.dma_start(out=cb2_sb[:, :], in_=cb2_t[:, :])
            s2_sb = cpool.tile([128, 6, 128], BF16, tag="s2")
            nc.scalar.dma_start(out=s2_sb[:, :, :],
                                in_=s2_t[:, :, :].rearrange("t p m -> p t m"))

            w2c_sb = cpool.tile([128, 4, NOUT], F32, tag="w2c")
            nc.scalar.dma_start(out=w2c_sb[:, :, :], in_=w2c_t[:, :, :])
            ident4_sb = cpool.tile([4, 4], F32, tag="id4")
            nc.scalar.dma_start(out=ident4_sb[:, :], in_=ident4_t[:, :])
            b2t_sb = cpool.tile([4, 4], F32, tag="b2t")
            nc.scalar.dma_start(out=b2t_sb[:, :], in_=b2t_t[:, :])

            # fc1 weight stream: 13 chunked DMAs, each gated on conv
            # progress via a 1-element ACT write into its destination
            # (WAW dep) — weights (needed only by fc1, ~100 us in) drain
            # paced by conv instead of starving the conv-critical
            # x9/repack traffic.  Chunk DMAs are emitted inside the conv
            # loops; see _wchunk().
            WCH = 8                         # k-tiles per chunk
            wch_edges = list(range(0, NKB, WCH)) + [NKB]
            wts = pers.tile([128, NKB, H1], FP8, tag="w1")
            wflat = wts[:, :, :].rearrange("p k m -> p (k m)")

            def _wchunk(i, gate_src):
                if i >= len(wch_edges) - 1:
                    return
                k0, k1 = wch_edges[i], wch_edges[i + 1]
                nc.scalar.activation(wts[0:1, k0, 0:1], gate_src, COPY)
                nc.sync.dma_start(out=wflat[:, H1 * k0:H1 * k1],
                                  in_=w1t_t[:, H1 * k0:H1 * k1])

            # align the 8 cores up front (PJRT launch skew would otherwise
            # surface as a long wait inside the first data collective);
            # the conv phase runs during the alignment, absorbing it.
            nc.gpsimd.collective_compute(
                "ReduceScatter", mybir.AluOpType.add,
                replica_groups=groups,
                ins=[sync_in[:, :]], outs=[sync_out[:, :]])

            # conv2 input buffer: partition e*64 + img*16 + ch holds the
            # padded channel image, dx-shifted by e.  Repack fills rows
            # 1..112 full-width; only rows 0/113 need zeroing.
            c2in = pers.tile([128, 114, 117], BF16, tag="bigC")
            nc.gpsimd.memset(c2in[:, 0, :], 0.0)
            nc.gpsimd.memset(c2in[:, 113, :], 0.0)

            # pool1 rows are 117 wide with zeroed borders (cols 0,
            # 113-116 + one spare element) so the repack shifts by e via
            # a single contiguous flat copy per (chunk, half, e).
            pool1_a = pers.tile([128, 28 * 117 + 1], BF16, tag="bigB1")
            pool1_b = pers.tile([128, 28 * 117 + 1], BF16, tag="bigB2")
            pool1_parts = [pool1_a, pool1_b]
            pool1_views = []
            for t in pool1_parts:
                pv = t[:, 0:28 * 117].rearrange("p (r c) -> p r c", c=117)
                nc.gpsimd.memset(pv[:, :, 0], 0.0)
                nc.gpsimd.memset(pv[:, :, 113:117], 0.0)
                nc.gpsimd.memset(t[:, 28 * 117:], 0.0)
                pool1_views.append(pv)

            # fc1 bias rides k-tile 98: ones column on partition 0
            hT = pers.tile([128, NKB, 32], BF16, tag="bigHT")
            nc.gpsimd.memset(hT[:, NK, :], 0.0)
            nc.gpsimd.memset(hT[0:1, NK, :], 1.0)

            # ---------------- conv1 + pool1 + relu ----------------
            # out partition m = h*16 + oc = half*64 + img*16 + oc
            for T in range(28):            # 2 pooled rows per psum tile
                ps = pspool.tile([128, 2, 512], F32, tag="ps")
                # de-interleave the 2x2 pool pairs in the PSUM AP: matmul
                # column j = (r, x, w) lands at offset 224r + 112w + x,
                # so the pool chain reads contiguous 112-runs.
                v = ps[:, :, 0:448].rearrange("p g (r w x) -> p g r x w",
                                              r=2, w=2)
                for g in range(2):
                    yp = T * 2 + g         # pooled row within half
                    e, ypl = yp // 4, yp % 4
                    rhs = x9_tiles[e][:, 2 * ypl:2 * ypl + 2,
                                      :224].rearrange(
                        "p r (x w) -> p r w x", w=2)
                    nc.tensor.matmul(
                        ps[:, g, 0:448],
                        lhsT=s1_sb[:, :],
                        rhs=rhs,
                        start=True, stop=True)
                c1 = wkpool.tile([128, 2, 2, 112], F32, tag="mc")
                nc.scalar.activation(c1[:, :, :, :], v[:, :, :, :, 1], COPY)
                m1 = wkpool.tile([128, 2, 2, 112], F32, tag="mx")
                nc.vector.tensor_max(m1[:, :, :, :], v[:, :, :, :, 0],
                                     c1[:, :, :, :])
                m2 = wkpool.tile([128, 2, 112], F32, tag="mxb")
                nc.vector.tensor_max(m2[:, :, :], m1[:, :, 0, :],
                                     m1[:, :, 1, :])
                half_t, row_t = divmod(2 * T, 28)
                nc.scalar.activation(
                    pool1_views[half_t][:, row_t:row_t + 2, 1:113],
                    m2[:, :, :], RELU, bias=cb1_sb[:, :])

            if stop_after == "conv1":
                dbg = wkpool.tile([4, NOUT], F32, tag="outsb")
                nc.vector.tensor_copy(dbg[:, :], pool1_views[0][0:4, 0, 1:5])
                nc.sync.dma_start(out=out_t[:, :], in_=dbg[:, :])
                return

            # -------- repack pool1 -> conv2 input (padded, merged halves,
            # two dx-shifted copies); 8 flat contiguous DMAs (one
            # descriptor per partition).  c2in[R0+r, c] = pool[r, c+e];
            # the pool's zeroed borders supply the conv padding.
            c2flat = c2in[:, :, :].rearrange("p r c -> p (r c)")
            for chunk in range(2):
                for half in range(2):
                    r0 = 117 * (56 * half + 1 + 28 * chunk)
                    for e in range(2):
                        nc.scalar.dma_start(
                            out=c2flat[64 * e:64 * e + 64,
                                       r0:r0 + 28 * 117],
                            in_=pool1_parts[chunk][64 * half:64 * half + 64,
                                                   e:e + 28 * 117])

            # ---------------- conv2 + pool2 + relu ----------------
            # out partition m = img*32 + oc; 6 passes t=(dy, grp):
            # partition block e supplies tap dx = 2*grp + e.
            h_sb = pers.tile([128, 7, 4, 2, 56], BF16, tag="bigD")
            for T in range(14):            # 8 conv rows / 4 pooled rows
                ps = pspool.tile([128, 2, 512], F32, tag="ps")
                # de-interleaved pool pairs: column j = (r, x, w) lands at
                # offset 112r + 56w + x (contiguous 56-runs for the pool).
                v = ps[:, :, 0:448].rearrange("p s (r w x) -> p s r x w",
                                              r=4, w=2)
                for sub in range(2):
                    y0 = 8 * T + 4 * sub
                    for t in range(6):
                        dy, grp = t // 2, t % 2
                        rhs = c2in[:, y0 + dy:y0 + dy + 4,
                                   2 * grp:2 * grp + 112].rearrange(
                            "p r (x w) -> p r w x", w=2)
                        nc.tensor.matmul(
                            ps[:, sub, 0:448],
                            lhsT=s2_sb[:, t, :],
                            rhs=rhs,
                            start=(t == 0), stop=(t == 5))
                c1 = wkpool.tile([128, 2, 4, 56], F32, tag="mc2")
                nc.scalar.activation(c1[:, :, :, :], v[:, :, :, :, 1], COPY)
                m1 = wkpool.tile([128, 2, 4, 56], F32, tag="mx2")
                nc.vector.tensor_max(m1[:, :, :, :], v[:, :, :, :, 0],
                                     c1[:, :, :, :])
                v2 = m1[:, :, :, :].rearrange("p s (rp w) x -> p s rp w x",
                                              w=2)
                m2 = wkpool.tile([128, 2, 2, 56], F32, tag="mxb2")
                nc.vector.tensor_max(m2[:, :, :, :], v2[:, :, :, 0, :],
                                     v2[:, :, :, 1, :])
                # pooled rows 4T..4T+4 -> h_sb[T//2, 2*(T%2) + (0..1), ...]
                nc.scalar.activation(
                    h_sb[:, T // 2, 2 * (T % 2):2 * (T % 2) + 2, :, :],
                    m2[:, :, :, :], RELU, bias=cb2_sb[:, :])
                # weight chunks gated on conv2 progress (conv1/repack get
                # the DMA bandwidth until conv2 is consuming); 2 chunks
                # per T so all 13 are in flight by T6 — well before fc1.
                _wchunk(2 * T, h_sb[0:1, T // 2, 2 * (T % 2), 0, 0:1])
                _wchunk(2 * T + 1, h_sb[0:1, T // 2, 2 * (T % 2), 0, 0:1])
                if T == 7:
                    # pooled rows 0-31 (t 0..3) complete: ship phase 1 of
                    # the AllToAll while conv2 finishes rows 32-55.
                    hf1 = h_sb[:, 0:4, :, :, :].rearrange(
                        "(i m) t j r x -> i m (t j r x)", i=4)
                    for il in range(4):
                        nc.scalar.dma_start(
                            out=a2a_in1[:, :].rearrange(
                                "(d i) (c s) -> i d c s", d=8, c=4)[il],
                            in_=hf1[il])
                    nc.gpsimd.collective_compute(
                        "AllToAll", mybir.AluOpType.bypass,
                        replica_groups=groups,
                        ins=[a2a_in1[:, :]], outs=[a2a_out1[:, :, :]])

            if stop_after == "conv2":
                dbg = wkpool.tile([4, NOUT], F32, tag="outsb")
                nc.vector.tensor_copy(dbg[:, :], h_sb[0:4, 0, 0, 0, 0:4])
                nc.sync.dma_start(out=out_t[:, :], in_=dbg[:, :])
                return

            if stop_after == "dumph":
                nc.sync.dma_start(
                    out=out_t[:, :],
                    in_=h_sb[:, :, :, :, :].rearrange("p t j r x -> p (t j r x)"))
                return

            # -------- AllToAll phase 2: pooled rows 32-55 (t 4..6) -----
            hf2 = h_sb[:, 4:7, :, :, :].rearrange(
                "(i m) t j r x -> i m (t j r x)", i=4)
            for il in range(4):
                nc.scalar.dma_start(
                    out=a2a_in2[:, :].rearrange(
                        "(d i) (c s) -> i d c s", d=8, c=4)[il],
                    in_=hf2[il])
            nc.gpsimd.collective_compute(
                "AllToAll", mybir.AluOpType.bypass, replica_groups=groups,
                ins=[a2a_in2[:, :]], outs=[a2a_out2[:, :, :]])

            # -------- two xbar transpose-DMAs -> feature-major hT ------
            # out[p, k, i] = in[i, 128k + p] per phase; k-tiles 0..55 are
            # phase-1 features (c, y<32, x), 56..97 phase-2.
            nc.scalar.dma_start(
                out=hT[:, 0:NK1, :],
                in_=a2a_out1[:, :, :].rearrange("s i f -> (s i) f"),
                transpose=True)
            nc.scalar.dma_start(
                out=hT[:, NK1:NK, :],
                in_=a2a_out2[:, :, :].rearrange("s i f -> (s i) f"),
                transpose=True)

            if stop_after == "a2a":
                dbg = wkpool.tile([4, NOUT], F32, tag="outsb")
                nc.vector.tensor_copy(dbg[:, :], hT[0:4, 0, 0:4])
                nc.sync.dma_start(out=out_t[:, :], in_=dbg[:, :])
                return

            if stop_after in ("dumphT", "dumpw"):
                src = hT if stop_after == "dumphT" else wts
                n = 32 if stop_after == "dumphT" else H1
                for k in range(NKB):
                    dbg = wkpool.tile([128, n], F32, tag="dmp")
                    nc.vector.tensor_copy(dbg[:, :], src[:, k, :])
                    nc.sync.dma_start(out=out_t[:, k, :], in_=dbg[:, :])
                return

            # ---------------- fc1 partial ----------------
            # 4 concurrent matmuls in 32-column PE groups (col tiling):
            # col group j accumulates k-tiles j, j+4, j+8, ... into PSUM
            # rows 32j..32j+32; the 4 row blocks are summed afterwards.
            fc1_ps = pspool.tile([128, H1], F32, tag="ps")
            NG = (NKB + 3) // 4
            for g in range(NG):
                for j in range(4):
                    k = 4 * g + j
                    if k >= NKB:
                        continue
                    nc.tensor.matmul(fc1_ps[32 * j:32 * j + 32, :],
                                     lhsT=hT[:, k, :], rhs=wts[:, k, :],
                                     tile_position=(0, 32 * j),
                                     start=(g == 0),
                                     stop=(k + 4 >= NKB))
            pa = wkpool.tile([B, H1], F32, tag="fc1a")
            nc.scalar.activation(pa[:, :], fc1_ps[32:64, :], COPY)
            pb = wkpool.tile([B, H1], F32, tag="fc1b")
            nc.scalar.activation(pb[:, :], fc1_ps[96:128, :], COPY)
            sa = wkpool.tile([B, H1], F32, tag="fc1c")
            nc.vector.tensor_add(sa[:, :], fc1_ps[0:32, :], pa[:, :])
            sb = wkpool.tile([B, H1], F32, tag="fc1d")
            nc.vector.tensor_add(sb[:, :], fc1_ps[64:96, :], pb[:, :])
            fc1_sb = wkpool.tile([B, H1], F32, tag="fc1")
            nc.vector.tensor_add(fc1_sb[:, :], sa[:, :], sb[:, :])
            nc.scalar.activation(fc1_sb[:, :], fc1_sb[:, :], COPY,
                                 scale=1.0 / W1SCALE)
            nc.scalar.dma_start(out=rs_in[:, :], in_=fc1_sb[:, :])

            if stop_after == "fc1":
                nc.sync.dma_start(out=out_t[:, :], in_=fc1_sb[0:4, 0:4])
                return

            # -------- ReduceScatter + relu + fc2 --------
            nc.gpsimd.collective_compute(
                "ReduceScatter", mybir.AluOpType.add, replica_groups=groups,
                ins=[rs_in[:, :]], outs=[rs_out[:, :]])

            h1 = wkpool.tile([4, H1], F32, tag="h1")
            nc.scalar.dma_start(out=h1[:, :], in_=rs_out[:, :])
            nc.scalar.activation(h1[:, :], h1[:, :], RELU)

            if stop_after == "rs":
                dbg = wkpool.tile([4, NOUT], F32, tag="outsb")
                nc.vector.tensor_copy(dbg[:, :], h1[0:4, 0:4])
                nc.sync.dma_start(out=out_t[:, :], in_=dbg[:, :])
                return

            # fc2 on the PE: transpose h1 into [128, 4kb, 4i] via 4 PE
            # transposes, then 4 accumulating [128,4i]x[128,4o] matmuls.
            h1t_ps = pspool.tile([128, 4, 4], F32, tag="ps")
            for kb in range(4):
                nc.tensor.transpose(h1t_ps[:, kb, :],
                                    h1[:, 128 * kb:128 * kb + 128],
                                    ident4_sb[:, :])
            h1t = wkpool.tile([128, 4, 4], F32, tag="h1t")
            nc.vector.tensor_copy(h1t[:, :, :], h1t_ps[:, :, :])
            fc2_ps = pspool.tile([4, NOUT], F32, tag="ps")
            for kb in range(4):
                nc.tensor.matmul(fc2_ps[:, :], lhsT=h1t[:, kb, :],
                                 rhs=w2c_sb[:, kb, :],
                                 start=(kb == 0), stop=(kb == 3))
            out_sb = wkpool.tile([4, NOUT], F32, tag="outsb")
            nc.vector.tensor_add(out_sb[:, :], fc2_ps[:, :], b2t_sb[:, :])
            nc.scalar.dma_start(out=out_t[:, :], in_=out_sb[:, :])


def _get_program():
    key = ("prog",)
    if key not in _CACHE:
        _CACHE[key] = _build_program()
    return _CACHE[key]


def _host_prep(x, conv1_w, conv1_b, conv2_w, conv2_b, values, w_idx1,
               fc1_b, w_idx2, fc2_b):
    """Build per-core input maps (numpy, bf16 for PE-facing tensors)."""
    f32 = np.float32
    x = np.asarray(x, f32)
    conv1_w = np.asarray(conv1_w, f32)
    conv2_w = np.asarray(conv2_w, f32)
    values = np.asarray(values, f32)
    w_idx1 = np.asarray(w_idx1)
    w_idx2 = np.asarray(w_idx2)

    x_pad = np.zeros((B, 226, 232), f32)
    x_pad[:, 1:225, 1:225] = x[:, 0]

    # x9[c]: [72, 112, 232]; partition (dy*3+dx)*8 + h, h = 4*half + img_loc
    x9 = np.zeros((N_CORES, 72, PH, 232), f32)
    for dy in range(3):
        for dx in range(3):
            for h in range(8):
                half, il = h // 4, h % 4
                y0 = PH * half
                for c in range(N_CORES):
                    x9[c, (dy * 3 + dx) * 8 + h, :, :232 - dx] = \
                        x_pad[4 * c + il, y0 + dy:y0 + dy + PH, dx:]

    s1 = np.zeros((72, 128), f32)
    for dy in range(3):
        for dx in range(3):
            for h in range(8):
                s1[(dy * 3 + dx) * 8 + h, 16 * h:16 * h + C1] = \
                    conv1_w[:, 0, dy, dx]

    # conv2 stationaries [6, 128, 128]: pass t = dy*2 + grp;
    # partition p = e*64 + img*16 + ch supplies tap dx = 2*grp + e
    s2 = np.zeros((6, 128, 128), f32)
    for t in range(6):
        dy, grp = t // 2, t % 2
        for e in range(2):
            dx = 2 * grp + e
            if dx > 2:
                continue
            for img in range(4):
                for ch in range(C1):
                    s2[t, 64 * e + 16 * img + ch, 32 * img:32 * img + C2] = \
                        conv2_w[:, ch, dy, dx]

    # fc1 weight [128, 99, 512] in fp8e4m3 scaled by W1SCALE: tile k
    # row p = feature 128k+p of the core's shard; tile 98 row 0 =
    # fc1_b/8 (ones-column bias trick; rides the same scale).
    b1_8 = np.asarray(fc1_b, f32) / N_CORES
    # feature order matches the two a2a phases: per channel, rows y<32
    # first (phase 1), then rows 32-55 (phase 2)
    perm = np.concatenate(
        [cl * 3136 + np.arange(32 * 56) for cl in range(4)]
        + [cl * 3136 + 32 * 56 + np.arange(24 * 56) for cl in range(4)])
    w1ts = []
    for c in range(N_CORES):
        idx = w_idx1[:, FSH * c:FSH * (c + 1)]             # [512, 12544]
        wt = np.zeros((128, NKB, H1), f32)
        wt[:, :NK, :] = values[idx].T[perm].reshape(
            NK, 128, H1).transpose(1, 0, 2)
        wt[0, NK, :] = b1_8
        wq = np.clip(wt * W1SCALE, -224.0, 224.0)
        w1ts.append(np.ascontiguousarray(
            wq.reshape(128, NKB * H1)).astype(ml_dtypes.float8_e4m3fn))

    w2 = np.ascontiguousarray(values[w_idx2]).astype(f32)     # [4, 512]
    # w2c[p, kb, o] = W2[o, 128*kb + p]
    w2c = np.ascontiguousarray(w2.T.reshape(4, 128, 4).transpose(1, 0, 2))
    ident4 = np.eye(4, dtype=f32)
    b2t = np.broadcast_to(np.asarray(fc2_b, f32), (4, 4)).copy()

    cb1 = np.zeros((128, 1), f32)
    for h in range(8):
        cb1[16 * h:16 * h + C1, 0] = np.asarray(conv1_b, f32)
    cb2 = np.zeros((128, 1), f32)
    for img in range(4):
        cb2[32 * img:32 * img + C2, 0] = np.asarray(conv2_b, f32)

    s1 = s1.astype(BF16NP)
    s2 = s2.astype(BF16NP)
    in_maps = []
    for c in range(N_CORES):
        in_maps.append({
            "x9": np.ascontiguousarray(x9[c]).astype(BF16NP),
            "s1": s1, "s2": s2,
            "w1t": w1ts[c],
            "w2c": w2c, "ident4": ident4, "b2t": b2t,
            "cb1": cb1, "cb2": cb2,
        })
    return in_maps


def kernel(x, conv1_w, conv1_b, conv2_w, conv2_b, values, w_idx1, fc1_b,
           w_idx2, fc2_b, _trace=False, _trace_kwargs=None):
    nc = _get_program()
    in_maps = _host_prep(x, conv1_w, conv1_b, conv2_w, conv2_b, values,
                         w_idx1, fc1_b, w_idx2, fc2_b)
    res = run_bass_kernel_spmd(nc, in_maps, core_ids=list(range(N_CORES)),
                               trace=_trace, **(_trace_kwargs or {}))
    out = np.zeros((B, NOUT), np.float32)
    for c in range(N_CORES):
        out[4 * c:4 * c + 4] = res.results[c]["out"]
    if _trace:
        kernel.last_result = res
    return out


if __name__ == "__main__":
    rng = np.random.default_rng(0)
    ins = {
        "x": rng.standard_normal((B, 1, IMG, IMG), dtype=np.float32),
        "conv1_w": rng.standard_normal((16, 1, 3, 3), dtype=np.float32) * 0.1,
        "conv1_b": np.zeros(16, np.float32),
        "conv2_w": rng.standard_normal((32, 16, 3, 3), dtype=np.float32) * 0.05,
        "conv2_b": np.zeros(32, np.float32),
        "values": np.sort(rng.standard_normal(4096).astype(np.float32) * 0.01),
        "w_idx1": rng.integers(0, 4096, (512, FEAT), dtype=np.int32),
        "fc1_b": np.zeros(512, np.float32),
        "w_idx2": rng.integers(0, 4096, (4, 512), dtype=np.int32),
        "fc2_b": np.zeros(4, np.float32),
    }
    out = kernel(**ins)
    print("out shape", out.shape, "sample row", out[0])
